# revision 1
# baseline (speedup 1.0000x reference)
"""Deformable 2D feature aggregator — Trainium2 Bass kernel, 8-core SPMD.

Problem: B=2, C=128, H=96, W=160, P=9 points, G=8 groups.
  value = conv1x1(feats); w = softmax over P of conv1x1(feats); offs = conv1x1(feats)
  pts = anchors + offs; out_proj(conv-weighted bilinear gather of value at pts).

Sharding: 8 cores = 2 batches x 4 query-slices. Each core computes the full
value map for its batch (cheap PE work), writes it bf16 to a DRAM scratch in
*rotated* pixel order (rotation = its query-slice offset, so the program is
identical across cores), then pair-gathers (x0,x0+1) channel rows with
dma_gather and does the bilinear+softmax-weighted reduction in query-major
layout on DVE/ACT with step-0 free-dim broadcasts.
"""
import sys

sys.path.insert(0, "/opt/trn_rl_repo")

import numpy as np
import ml_dtypes

import concourse.bass as bass
import concourse.bacc as bacc
import concourse.mybir as mybir
import concourse.tile as tile
from concourse import library_config
from concourse.ap import AP

# problem constants (hardcoded per harness contract)
B, C, H, W = 2, 128, 96, 160
HW = H * W                     # 15360
P, G, GC = 9, 8, 16
NCORES = 8
QS = B * HW // NCORES          # 3840 queries per core
NT = QS // 128                 # 30 query tiles
TCH = 2                        # query tiles per gather chunk
NCH = NT // TCH                # 15 gather chunks
NJ = 2 * P                     # 18 row-gathers (pairs) per query
NIDX_CH = TCH * 128 * NJ       # 4608 gather indices per chunk
SHIFT = 1024.0                 # floor-bias (exact in f32 for our range)
FCHUNK = 1920                  # feats DMA chunk (pixels)
NPXT = HW // 128               # 120 pixel tiles

f32 = mybir.dt.float32
bf16 = mybir.dt.bfloat16
i16 = mybir.dt.int16
Alu = mybir.AluOpType
Act = mybir.ActivationFunctionType
Ax = mybir.AxisListType

_CACHE: dict = {}


def _build_nc(stage=None):
    import os
    stage = stage or os.environ.get("BASS_STAGE", "full")
    nc = bacc.Bacc()

    feats = nc.dram_tensor("feats", [C, HW], f32, kind="ExternalInput")
    anch = nc.dram_tensor("anch", [128, NT * 2], f32, kind="ExternalInput")
    vwT = nc.dram_tensor("vwT", [C, C], f32, kind="ExternalInput")
    w90T = nc.dram_tensor("w90T", [C, 90], f32, kind="ExternalInput")
    owT = nc.dram_tensor("owT", [C, C], f32, kind="ExternalInput")
    b90r = nc.dram_tensor("b90r", [128, 90], f32, kind="ExternalInput")
    bvr = nc.dram_tensor("bvr", [128, C], f32, kind="ExternalInput")
    outb = nc.dram_tensor("outb", [128, 1], f32, kind="ExternalInput")
    oneh = nc.dram_tensor("oneh", [128, 8 * 128], f32, kind="ExternalInput")
    ident = nc.dram_tensor("ident", [128, 128], f32, kind="ExternalInput")
    rotoff = nc.dram_tensor("rotoff", [128, 1], f32, kind="ExternalInput")
    out_d = nc.dram_tensor("out", [C, QS], f32, kind="ExternalOutput")

    with tile.TileContext(nc) as tc, nc.allow_low_precision("bf16 combine by design"):
        with (
            tc.tile_pool(name="const", bufs=1) as cpool,
            tc.tile_pool(name="stage", bufs=1) as spool,
            tc.tile_pool(name="feats", bufs=2) as fpool,
            tc.tile_pool(name="vsb", bufs=3) as vpool,
            tc.tile_pool(name="g", bufs=2) as gpool,
            tc.tile_pool(name="tprime", bufs=2) as tpool,
            tc.tile_pool(name="aggp", bufs=2) as apool,
            tc.tile_pool(name="ps", bufs=1, space="PSUM") as pspool,
            tc.tile_pool(name="dram", bufs=1, space="DRAM") as dpool,
        ):
            # ---- persistent loads ----
            vwT_s = cpool.tile([C, C], f32)
            nc.sync.dma_start(vwT_s[:], vwT[:])
            w90T_s = cpool.tile([C, 90], f32)
            nc.sync.dma_start(w90T_s[:], w90T[:])
            owT_s = cpool.tile([C, C], f32)
            nc.sync.dma_start(owT_s[:], owT[:])
            b90_s = cpool.tile([128, 90], f32)
            nc.sync.dma_start(b90_s[:], b90r[:])
            bvr_s = cpool.tile([128, C], f32)
            nc.sync.dma_start(bvr_s[:], bvr[:])
            outb_s = cpool.tile([128, 1], f32)
            nc.sync.dma_start(outb_s[:], outb[:])
            oneh_s = cpool.tile([128, 8 * 128], f32)
            nc.sync.dma_start(oneh_s[:], oneh[:])
            ident_s = cpool.tile([128, 128], f32)
            nc.sync.dma_start(ident_s[:], ident[:])
            anch_s = cpool.tile([128, NT * 2], f32)
            nc.sync.dma_start(anch_s[:], anch[:])
            rot_s = cpool.tile([128, 1], f32)
            nc.sync.dma_start(rot_s[:], rotoff[:])

            # +1 pad row backs the last row's pair window; rotated indices
            # never exceed HW-2 (xg<=W-2), so the pad row is never read with
            # nonzero weight — zero it to keep every readable byte finite.
            valscr = dpool.tile([HW + 1, C], bf16)
            zrow = cpool.tile([1, C], bf16)
            nc.vector.memset(zrow[:], 0.0)
            nc.sync.dma_start(valscr[HW : HW + 1, :], zrow[:])

            # staging tiles
            proj_s = spool.tile([128, NT * 90], f32)
            FO = spool.tile([128, QS], f32)
            if stage != "full":
                nc.vector.memset(FO[:], 0.0)

            # ---- phase A: value projection over the whole (rotated) image ----
            fch = None
            for t in range(NPXT):
                if t % (FCHUNK // 128) == 0:
                    fch = fpool.tile([128, FCHUNK], f32)
                    nc.sync.dma_start(fch[:], feats[:, t * 128 : t * 128 + FCHUNK])
                col = (t % (FCHUNK // 128)) * 128
                lhsT = fch[:, col : col + 128]
                vps = pspool.tile([128, C], f32, tag="v", bufs=2, name=f"vps{t}")
                nc.tensor.matmul(vps[:], lhsT, vwT_s[:], start=True, stop=True)
                vsb = vpool.tile([128, C], bf16, tag="vsb", name=f"vsb{t}")
                nc.scalar.copy(vsb[:], vps[:])
                nc.sync.dma_start(valscr[t * 128 : (t + 1) * 128, :], vsb[:])
                if t < NT:
                    pps = pspool.tile([128, 90], f32, tag="p", bufs=2, name=f"pps{t}")
                    nc.tensor.matmul(pps[:], lhsT, w90T_s[:], start=True, stop=True)
                    nc.vector.tensor_tensor(
                        out=proj_s[:, t * 90 : (t + 1) * 90],
                        in0=pps[:],
                        in1=b90_s[:],
                        op=Alu.add,
                    )

            # ---- phase B: batched softmax / coords / weights (query-major) ----
            # proj_s free layout per tile t: [0,72) = wlog (pt*8+g), [72,90) = offs (pt*2+xy)
            pv = proj_s[:, :].rearrange("p (t k) -> p t k", k=90)

            # softmax over points
            wmax = spool.tile([128, NT * G], f32)
            wl_gp = AP(tensor=proj_s.tensor, offset=proj_s[:, :].offset,
                       ap=[[proj_s[:, :].ap[0][0], 128], [90, NT], [1, G], [G, P]])
            nc.vector.tensor_reduce(out=wmax[:, :].rearrange("p (t g) -> p t g", g=G),
                                    in_=wl_gp, axis=Ax.X, op=Alu.max)
            smf = spool.tile([128, NT * P * G], f32)
            wl_pg = AP(tensor=proj_s.tensor, offset=proj_s[:, :].offset,
                       ap=[[proj_s[:, :].ap[0][0], 128], [90, NT], [G, P], [1, G]])
            wmax_b = AP(tensor=wmax.tensor, offset=wmax[:, :].offset,
                        ap=[[wmax[:, :].ap[0][0], 128], [G, NT], [0, P], [1, G]])
            nc.vector.tensor_tensor(
                out=smf[:, :].rearrange("p (t q g) -> p t q g", q=P, g=G),
                in0=wl_pg, in1=wmax_b, op=Alu.subtract)
            nc.scalar.activation(smf[:], smf[:], Act.Exp)
            ssum = spool.tile([128, NT * G], f32)
            sm_gp = AP(tensor=smf.tensor, offset=smf[:, :].offset,
                       ap=[[smf[:, :].ap[0][0], 128], [P * G, NT], [1, G], [G, P]])
            nc.vector.tensor_reduce(out=ssum[:, :].rearrange("p (t g) -> p t g", g=G),
                                    in_=sm_gp, axis=Ax.X, op=Alu.add)
            rcps = spool.tile([128, NT * G], f32)
            nc.vector.reciprocal(rcps[:], ssum[:])
            wsm = spool.tile([128, NT * P * G], bf16)
            rcp_b = AP(tensor=rcps.tensor, offset=rcps[:, :].offset,
                       ap=[[rcps[:, :].ap[0][0], 128], [G, NT], [0, P], [1, G]])
            nc.vector.tensor_tensor(
                out=wsm[:, :].rearrange("p (t q g) -> p t q g", q=P, g=G),
                in0=smf[:, :].rearrange("p (t q g) -> p t q g", q=P, g=G),
                in1=rcp_b, op=Alu.mult)

            # coords: px/py [128, NT*P] laid out (t, pt)
            NP_ = NT * P
            def sap(tl, dims):
                a = tl[:, :] if not isinstance(tl, AP) else tl
                return AP(tensor=a.tensor, offset=a.offset, ap=[list(a.ap[0])] + dims)

            px = spool.tile([128, NP_], f32)
            py = spool.tile([128, NP_], f32)
            offs_x = AP(tensor=proj_s.tensor, offset=proj_s[:, :].offset + 72,
                        ap=[[proj_s[:, :].ap[0][0], 128], [90, NT], [2, P]])
            offs_y = AP(tensor=proj_s.tensor, offset=proj_s[:, :].offset + 73,
                        ap=[[proj_s[:, :].ap[0][0], 128], [90, NT], [2, P]])
            anx = AP(tensor=anch_s.tensor, offset=anch_s[:, :].offset,
                     ap=[[anch_s[:, :].ap[0][0], 128], [2, NT], [0, P]])
            any_ = AP(tensor=anch_s.tensor, offset=anch_s[:, :].offset + 1,
                      ap=[[anch_s[:, :].ap[0][0], 128], [2, NT], [0, P]])
            pxv = px[:, :].rearrange("p (t q) -> p t q", q=P)
            pyv = py[:, :].rearrange("p (t q) -> p t q", q=P)
            nc.vector.tensor_tensor(out=pxv, in0=offs_x, in1=anx, op=Alu.add)
            nc.vector.tensor_tensor(out=pyv, in0=offs_y, in1=any_, op=Alu.add)

            xp = spool.tile([128, NP_], f32)
            yp = spool.tile([128, NP_], f32)
            nc.scalar.activation(xp[:], px[:], Act.Copy, bias=SHIFT - 0.5, scale=float(W))
            nc.scalar.activation(yp[:], py[:], Act.Copy, bias=SHIFT - 0.5, scale=float(H))
            # floor via round(x-0.5): (x + (2^23-0.5)) - 2^23. At integer x the
            # half-even tie may floor one low with frac 1.0 — an equivalent
            # bilinear weighting, so interpolation is unchanged.
            MAGIC = float(1 << 23)
            xf = spool.tile([128, NP_], f32)
            yf = spool.tile([128, NP_], f32)
            nc.vector.tensor_scalar(out=xf[:], in0=xp[:], scalar1=MAGIC - 0.5,
                                    scalar2=MAGIC, op0=Alu.add, op1=Alu.subtract)
            nc.vector.tensor_scalar(out=yf[:], in0=yp[:], scalar1=MAGIC - 0.5,
                                    scalar2=MAGIC, op0=Alu.add, op1=Alu.subtract)
            wx = spool.tile([128, NP_], f32)
            wy = spool.tile([128, NP_], f32)
            nc.vector.tensor_tensor(out=wx[:], in0=xp[:], in1=xf[:], op=Alu.subtract)
            nc.vector.tensor_tensor(out=wy[:], in0=yp[:], in1=yf[:], op=Alu.subtract)

            xg = spool.tile([128, NP_], f32)
            nc.vector.tensor_scalar(out=xg[:], in0=xf[:], scalar1=SHIFT, scalar2=0.0,
                                    op0=Alu.subtract, op1=Alu.max)
            nc.vector.tensor_scalar(out=xg[:], in0=xg[:], scalar1=float(W - 2), scalar2=None, op0=Alu.min)
            yg0 = spool.tile([128, NP_], f32)
            nc.vector.tensor_scalar(out=yg0[:], in0=yf[:], scalar1=SHIFT, scalar2=0.0,
                                    op0=Alu.subtract, op1=Alu.max)
            nc.vector.tensor_scalar(out=yg0[:], in0=yg0[:], scalar1=float(H - 1), scalar2=None, op0=Alu.min)
            yg1 = spool.tile([128, NP_], f32)
            nc.vector.tensor_scalar(out=yg1[:], in0=yf[:], scalar1=SHIFT - 1.0, scalar2=0.0,
                                    op0=Alu.subtract, op1=Alu.max)
            nc.vector.tensor_scalar(out=yg1[:], in0=yg1[:], scalar1=float(H - 1), scalar2=None, op0=Alu.min)

            # x-validity masks (with pair-clamp weight swap)
            tA = spool.tile([128, NP_], f32)
            tB = spool.tile([128, NP_], f32)
            mA = spool.tile([128, NP_], f32)
            nc.vector.tensor_scalar(out=tA[:], in0=xf[:], scalar1=SHIFT, scalar2=None, op0=Alu.is_ge)
            nc.vector.tensor_scalar(out=tB[:], in0=xf[:], scalar1=SHIFT + W - 2, scalar2=None, op0=Alu.is_le)
            nc.vector.tensor_tensor(out=mA[:], in0=tA[:], in1=tB[:], op=Alu.mult)
            mB = spool.tile([128, NP_], f32)
            nc.vector.tensor_scalar(out=mB[:], in0=xf[:], scalar1=SHIFT - 1.0, scalar2=None, op0=Alu.is_equal)
            mC = spool.tile([128, NP_], f32)
            nc.vector.tensor_scalar(out=mC[:], in0=xf[:], scalar1=SHIFT + W - 1, scalar2=None, op0=Alu.is_equal)
            ux = spool.tile([128, NP_], f32)
            uy = spool.tile([128, NP_], f32)
            nc.scalar.activation(ux[:], wx[:], Act.Copy, bias=1.0, scale=-1.0)
            nc.scalar.activation(uy[:], wy[:], Act.Copy, bias=1.0, scale=-1.0)

            bx = spool.tile([128, NT * P * 2], f32)   # (t, pt, side)
            v1 = spool.tile([128, NP_], f32)
            v2 = spool.tile([128, NP_], f32)
            nc.vector.tensor_tensor(out=v1[:], in0=ux[:], in1=mA[:], op=Alu.mult)
            nc.vector.tensor_tensor(out=v2[:], in0=wx[:], in1=mB[:], op=Alu.mult)
            bx0 = AP(tensor=bx.tensor, offset=bx[:, :].offset,
                     ap=[[bx[:, :].ap[0][0], 128], [2, NP_]])
            nc.vector.tensor_tensor(out=bx0, in0=v1[:], in1=v2[:], op=Alu.add)
            nc.vector.tensor_tensor(out=v1[:], in0=wx[:], in1=mA[:], op=Alu.mult)
            nc.vector.tensor_tensor(out=v2[:], in0=ux[:], in1=mC[:], op=Alu.mult)
            bx1 = AP(tensor=bx.tensor, offset=bx[:, :].offset + 1,
                     ap=[[bx[:, :].ap[0][0], 128], [2, NP_]])
            nc.vector.tensor_tensor(out=bx1, in0=v1[:], in1=v2[:], op=Alu.add)

            # y masks / weights
            my0 = spool.tile([128, NP_], f32)
            my1 = spool.tile([128, NP_], f32)
            nc.vector.tensor_scalar(out=tA[:], in0=yf[:], scalar1=SHIFT, scalar2=None, op0=Alu.is_ge)
            nc.vector.tensor_scalar(out=tB[:], in0=yf[:], scalar1=SHIFT + H - 1, scalar2=None, op0=Alu.is_le)
            nc.vector.tensor_tensor(out=my0[:], in0=tA[:], in1=tB[:], op=Alu.mult)
            nc.vector.tensor_scalar(out=tA[:], in0=yf[:], scalar1=SHIFT - 1.0, scalar2=None, op0=Alu.is_ge)
            nc.vector.tensor_scalar(out=tB[:], in0=yf[:], scalar1=SHIFT + H - 2, scalar2=None, op0=Alu.is_le)
            nc.vector.tensor_tensor(out=my1[:], in0=tA[:], in1=tB[:], op=Alu.mult)
            by = spool.tile([128, NT * P * 2], f32)   # (t, pt, row)
            by0 = AP(tensor=by.tensor, offset=by[:, :].offset,
                     ap=[[by[:, :].ap[0][0], 128], [2, NP_]])
            by1 = AP(tensor=by.tensor, offset=by[:, :].offset + 1,
                     ap=[[by[:, :].ap[0][0], 128], [2, NP_]])
            nc.vector.tensor_tensor(out=by0, in0=uy[:], in1=my0[:], op=Alu.mult)
            nc.vector.tensor_tensor(out=by1, in0=wy[:], in1=my1[:], op=Alu.mult)

            # gather row indices (rotated): idx = (y*W + x - rotoff_biased) mod HW
            idxf = spool.tile([128, NT * NJ], f32)    # (t, pt, row)
            r0 = spool.tile([128, NP_], f32)
            nc.scalar.activation(r0[:], yg0[:], Act.Copy, bias=0.0, scale=float(W))
            idx0 = AP(tensor=idxf.tensor, offset=idxf[:, :].offset,
                      ap=[[idxf[:, :].ap[0][0], 128], [2, NP_]])
            nc.vector.tensor_tensor(out=idx0, in0=r0[:], in1=xg[:], op=Alu.add)
            nc.scalar.activation(r0[:], yg1[:], Act.Copy, bias=0.0, scale=float(W))
            idx1 = AP(tensor=idxf.tensor, offset=idxf[:, :].offset + 1,
                      ap=[[idxf[:, :].ap[0][0], 128], [2, NP_]])
            nc.vector.tensor_tensor(out=idx1, in0=r0[:], in1=xg[:], op=Alu.add)
            # rotate into this core's value-map pixel order, wrapping mod HW
            nc.vector.tensor_scalar(out=idxf[:], in0=idxf[:], scalar1=rot_s[:, 0:1],
                                    scalar2=None, op0=Alu.subtract)
            wrap = spool.tile([128, NT * NJ], f32)
            nc.vector.tensor_scalar(out=wrap[:], in0=idxf[:], scalar1=0.0,
                                    scalar2=float(HW), op0=Alu.is_lt, op1=Alu.mult)
            nc.vector.tensor_tensor(out=idxf[:], in0=idxf[:], in1=wrap[:], op=Alu.add)

            # cw[t, pt, row, side] = by[t,pt,row] * bx[t,pt,side]  (bf16)
            cw = spool.tile([128, NT * P * 4], bf16)
            for row in range(2):
                by_r = AP(tensor=by.tensor, offset=by[:, :].offset + row,
                          ap=[[by[:, :].ap[0][0], 128], [2 * P, NT], [2, P], [0, 2]])
                bx_v = AP(tensor=bx.tensor, offset=bx[:, :].offset,
                          ap=[[bx[:, :].ap[0][0], 128], [2 * P, NT], [2, P], [1, 2]])
                cw_r = AP(tensor=cw.tensor, offset=cw[:, :].offset + 2 * row,
                          ap=[[cw[:, :].ap[0][0], 128], [4 * P, NT], [4, P], [1, 2]])
                nc.vector.tensor_tensor(out=cw_r, in0=by_r, in1=bx_v, op=Alu.mult)

            # kw[t, pt, rs, g] = cw[t, pt, rs] * wsm[t, pt, g]  (bf16)
            kw = spool.tile([128, NT * P * 4 * G], bf16)
            for rs in range(4):
                cw_rs = AP(tensor=cw.tensor, offset=cw[:, :].offset + rs,
                           ap=[[cw[:, :].ap[0][0], 128], [4 * P, NT], [4, P], [0, G]])
                w_v = AP(tensor=wsm.tensor, offset=wsm[:, :].offset,
                         ap=[[wsm[:, :].ap[0][0], 128], [P * G, NT], [G, P], [1, G]])
                kw_rs = AP(tensor=kw.tensor, offset=kw[:, :].offset + rs * G,
                           ap=[[kw[:, :].ap[0][0], 128], [4 * P * G, NT], [4 * G, P], [1, G]])
                nc.vector.tensor_tensor(out=kw_rs, in0=cw_rs, in1=w_v, op=Alu.mult)

            # sumcoef[t, g] = sum_pt wsm * (bx0+bx1)*(by0+by1)   (for value_b)
            bsx = spool.tile([128, NP_], f32)
            bsy = spool.tile([128, NP_], f32)
            bx0r = AP(tensor=bx.tensor, offset=bx[:, :].offset, ap=[[bx[:, :].ap[0][0], 128], [2, NP_]])
            bx1r = AP(tensor=bx.tensor, offset=bx[:, :].offset + 1, ap=[[bx[:, :].ap[0][0], 128], [2, NP_]])
            by0r = AP(tensor=by.tensor, offset=by[:, :].offset, ap=[[by[:, :].ap[0][0], 128], [2, NP_]])
            by1r = AP(tensor=by.tensor, offset=by[:, :].offset + 1, ap=[[by[:, :].ap[0][0], 128], [2, NP_]])
            nc.vector.tensor_tensor(out=bsx[:], in0=bx0r, in1=bx1r, op=Alu.add)
            nc.vector.tensor_tensor(out=bsy[:], in0=by0r, in1=by1r, op=Alu.add)
            bws = spool.tile([128, NP_], bf16)
            nc.vector.tensor_tensor(out=bws[:], in0=bsx[:], in1=bsy[:], op=Alu.mult)
            wp = spool.tile([128, NT * P * G], bf16)
            bws_b = AP(tensor=bws.tensor, offset=bws[:, :].offset,
                       ap=[[bws[:, :].ap[0][0], 128], [P, NT], [1, P], [0, G]])
            nc.vector.tensor_tensor(
                out=wp[:, :].rearrange("p (t q g) -> p t q g", q=P, g=G),
                in0=wsm[:, :].rearrange("p (t q g) -> p t q g", q=P, g=G),
                in1=bws_b, op=Alu.mult)
            sumcoef = spool.tile([128, NT * G], f32)
            wp_gp = AP(tensor=wp.tensor, offset=wp[:, :].offset,
                       ap=[[wp[:, :].ap[0][0], 128], [P * G, NT], [1, G], [G, P]])
            nc.vector.tensor_reduce(out=sumcoef[:, :].rearrange("p (t g) -> p t g", g=G),
                                    in_=wp_gp, axis=Ax.X, op=Alu.add)

            # ---- phase C: per-chunk idx16 build + gather + combine ----
            idx16 = spool.tile([128, NCH * (NIDX_CH // 16)], i16)
            val_src = AP(tensor=valscr.tensor, offset=valscr[:, :].offset,
                         ap=[[C, HW], [1, 2 * C]])

            n_ch = NCH if stage in ("full", "nogather") else int(stage)
            for ch in range(n_ch):
                # PE permutation: idxq[16qh+p16, j] -> i16psum[:, 8j+qh] (replicated x8)
                i16ps = pspool.tile([128, NIDX_CH // 16], f32, tag="i16", bufs=2, name=f"i16ps{ch}")
                for qh in range(8):
                    outap = AP(tensor=i16ps.tensor, offset=i16ps[:, :].offset + qh,
                               ap=[[i16ps[:, :].ap[0][0], 128], [8, TCH * NJ]])
                    nc.tensor.matmul(outap, oneh_s[:, qh * 128 : (qh + 1) * 128],
                                     idxf[:, ch * TCH * NJ : (ch + 1) * TCH * NJ],
                                     start=True, stop=True)
                nc.vector.tensor_copy(
                    idx16[:, ch * (NIDX_CH // 16) : (ch + 1) * (NIDX_CH // 16)], i16ps[:])

                gt = gpool.tile([128, TCH * NJ, 2 * C], bf16, tag="g", name=f"g{ch}")
                if stage == "nogather":
                    nc.vector.memset(gt[:, :, :], 0.0)
                else:
                    nc.gpsimd.dma_gather(
                    gt[:, :, :], val_src,
                    idx16[:, ch * (NIDX_CH // 16) : (ch + 1) * (NIDX_CH // 16)],
                        num_idxs=NIDX_CH, num_idxs_reg=NIDX_CH,
                        elem_size=2 * C, elem_step=C, single_packet=False,
                    )

                for tt_ in range(TCH):
                    t = ch * TCH + tt_
                    # T' = G * kw  with  G [q, (pt,rs), (g,gc)], kw bcast over gc
                    tp = tpool.tile([128, NJ * 2, C], bf16, tag="tp", name=f"tp{t}")
                    # free offset within gt for (pt,row,side,c) = (pt*4+row*2+side)*C + c
                    g_v = AP(tensor=gt.tensor,
                             offset=gt[:, :, :].offset + tt_ * NJ * 2 * C,
                             ap=[[gt[:, :, :].ap[0][0], 128], [C, NJ * 2], [GC, G], [1, GC]])
                    kw_v = AP(tensor=kw.tensor, offset=kw[:, :].offset + t * P * 4 * G,
                              ap=[[kw[:, :].ap[0][0], 128], [G, NJ * 2], [1, G], [0, GC]])
                    tp_v = AP(tensor=tp.tensor, offset=tp[:, :, :].offset,
                              ap=[[tp[:, :, :].ap[0][0], 128], [C, NJ * 2], [GC, G], [1, GC]])
                    nc.any.tensor_tensor(out=tp_v, in0=g_v, in1=kw_v, op=Alu.mult)

                    # agg[q, c] = sum over the 36 (pt,rs) terms
                    agg = apool.tile([128, C], f32, tag="agg", name=f"agg{t}")
                    tp_r = AP(tensor=tp.tensor, offset=tp[:, :, :].offset,
                              ap=[[tp[:, :, :].ap[0][0], 128], [1, C], [C, NJ * 2]])
                    nc.vector.tensor_reduce(out=agg[:], in_=tp_r, axis=Ax.X, op=Alu.add)

                    # + value_b * sumcoef  (per query, per group)
                    ebias = apool.tile([128, C], f32, tag="eb", name=f"eb{t}")
                    sc_v = AP(tensor=sumcoef.tensor, offset=sumcoef[:, :].offset + t * G,
                              ap=[[sumcoef[:, :].ap[0][0], 128], [1, G], [0, GC]])
                    bv_v = bvr_s[:, :].rearrange("p (g c) -> p g c", g=G)
                    nc.vector.tensor_tensor(out=ebias[:, :].rearrange("p (g c) -> p g c", g=G),
                                            in0=sc_v, in1=bv_v, op=Alu.mult)
                    agg2 = apool.tile([128, C], f32, tag="agg2", name=f"agg2{t}")
                    nc.vector.tensor_tensor(out=agg2[:], in0=agg[:], in1=ebias[:], op=Alu.add)

                    # transpose -> [c, q], out-projection, bias, stage to FO
                    trps = pspool.tile([128, C], f32, tag="tr", bufs=1, name=f"tr{t}")
                    nc.tensor.transpose(trps[:], agg2[:], ident_s[:])
                    aggT = apool.tile([128, C], f32, tag="aggT", name=f"aggT{t}")
                    nc.scalar.copy(aggT[:], trps[:])
                    fops = pspool.tile([128, C], f32, tag="fo", bufs=1, name=f"fo{t}")
                    nc.tensor.matmul(fops[:], owT_s[:], aggT[:], start=True, stop=True)
                    nc.scalar.activation(FO[:, t * 128 : (t + 1) * 128], fops[:],
                                         Act.Identity, bias=outb_s[:, 0:1], scale=1.0)

            nc.sync.dma_start(out_d[:], FO[:])

    nc.finalize()
    return nc


def _host_prep(inputs):
    """Prepare per-core input maps from full inputs."""
    feats = np.asarray(inputs["feats"], np.float32)          # [B, C, H, W]
    anchor = np.asarray(inputs["anchor_points"], np.float32)  # [B, HW, 2]
    value_w = np.asarray(inputs["value_w"], np.float32)
    value_b = np.asarray(inputs["value_b"], np.float32)
    weights_w = np.asarray(inputs["weights_w"], np.float32)
    weights_b = np.asarray(inputs["weights_b"], np.float32)
    offset_w = np.asarray(inputs["offset_w"], np.float32)
    offset_b = np.asarray(inputs["offset_b"], np.float32)
    out_w = np.asarray(inputs["out_w"], np.float32)
    out_b = np.asarray(inputs["out_b"], np.float32)

    w90 = np.concatenate([weights_w, offset_w], 0)            # [90, C]
    b90 = np.concatenate([weights_b, offset_b], 0)            # [90]
    shared = {
        "vwT": np.ascontiguousarray(value_w.T),
        "w90T": np.ascontiguousarray(w90.T),
        "owT": np.ascontiguousarray(out_w.T),
        "b90r": np.broadcast_to(b90, (128, 90)).copy(),
        "bvr": np.broadcast_to(value_b, (128, C)).copy(),
        "outb": out_b.reshape(128, 1).copy(),
        "ident": np.eye(128, dtype=np.float32),
    }
    oneh = np.zeros((128, 8, 128), np.float32)
    for qh in range(8):
        for m in range(128):
            oneh[16 * qh + (m % 16), qh, m] = 1.0
    shared["oneh"] = oneh.reshape(128, 8 * 128)

    in_maps = []
    for core in range(NCORES):
        b_i, sl = core // 4, core % 4
        off = sl * QS
        fr = np.roll(feats[b_i].reshape(C, HW), -off, axis=1)
        an = anchor[b_i, off : off + QS].reshape(NT, 128, 2).transpose(1, 0, 2).reshape(128, NT * 2)
        m = dict(shared)
        m["feats"] = np.ascontiguousarray(fr)
        m["anch"] = np.ascontiguousarray(an)
        m["rotoff"] = np.full((128, 1), float(off), np.float32)
        in_maps.append(m)
    return in_maps


def kernel(**inputs) -> np.ndarray:
    from concourse.bass_utils import run_bass_kernel_spmd

    if "nc" not in _CACHE:
        _CACHE["nc"] = _build_nc()
    nc = _CACHE["nc"]
    in_maps = _host_prep(inputs)
    res = run_bass_kernel_spmd(nc, in_maps, core_ids=list(range(NCORES)))
    out = np.zeros((B, C, HW), np.float32)
    for core in range(NCORES):
        b_i, sl = core // 4, core % 4
        out[b_i, :, sl * QS : (sl + 1) * QS] = res.results[core]["out"]
    return out.reshape(B, C, H, W)



# revision 3
# speedup vs baseline: 1.8216x; 1.8216x over previous
"""Deformable 2D feature aggregator — Trainium2 Bass kernel, 8-core SPMD. v2.

Problem: B=2, C=128, H=96, W=160, P=9 points, G=8 groups.
  value = conv1x1(feats); w = softmax over P of conv1x1(feats); offs = conv1x1(feats)
  pts = anchors + offs; out_proj(conv-weighted bilinear gather of value at pts).

Sharding: 8 cores = 2 batches x 4 query-slices; each core builds the full
(rotated) value map for its batch, then gathers per-query corner windows.

v2 vs v1:
  - pair-row DRAM scratch [HW+1, 2C]: record r = [V(r), V(r+W mod HW)], so ONE
    1024B dma_gather descriptor fetches all 4 bilinear corners of a point
    (halves the GPSIMD SWDGE desc-gen, which dominated the v1 trace at 545us).
  - value/out projections in bf16 on PE (4x); offsets stay f32 (precision).
  - combine: one contiguous 3D-AP multiply per tile (DVE), one 2x-mode pair
    pre-add, then 19 transpose-accumulate matmuls into PSUM (reduce+transpose
    fused on PE), out-projection straight off the accumulated PSUM.
"""
import sys

sys.path.insert(0, "/opt/trn_rl_repo")

import numpy as np
import ml_dtypes

import concourse.bass as bass
import concourse.bacc as bacc
import concourse.mybir as mybir
import concourse.tile as tile
from concourse import library_config
from concourse.ap import AP

# problem constants (hardcoded per harness contract)
B, C, H, W = 2, 128, 96, 160
HW = H * W                     # 15360
P, G, GC = 9, 8, 16
NCORES = 8
QS = B * HW // NCORES          # 3840 queries per core
NT = QS // 128                 # 30 query tiles
TCH = 2                        # query tiles per gather chunk
NCH = NT // TCH                # 15 gather chunks
NP_ = NT * P                   # 270 points per partition-row
NIDX_CH = TCH * 128 * P        # 2304 gather indices per chunk
SHIFT = 1024.0                 # floor-bias (exact in f32 for our range)
NPXT = HW // 128               # 120 pixel tiles
VT = 5                         # pixel tiles per valscr2 write chunk
NCHV = NPXT // VT              # 24 write chunks

f32 = mybir.dt.float32
bf16 = mybir.dt.bfloat16
i16 = mybir.dt.int16
Alu = mybir.AluOpType
Act = mybir.ActivationFunctionType
Ax = mybir.AxisListType

_CACHE: dict = {}


def _build_nc():
    nc = bacc.Bacc()

    feats16 = nc.dram_tensor("feats16", [C, HW], bf16, kind="ExternalInput")
    feats32 = nc.dram_tensor("feats32", [C, QS], f32, kind="ExternalInput")
    anch = nc.dram_tensor("anch", [128, NT * 2], f32, kind="ExternalInput")
    vwT16 = nc.dram_tensor("vwT16", [C, C], bf16, kind="ExternalInput")
    w90T = nc.dram_tensor("w90T", [C, 90], f32, kind="ExternalInput")
    owT16 = nc.dram_tensor("owT16", [C, C], bf16, kind="ExternalInput")
    b90r = nc.dram_tensor("b90r", [128, 90], f32, kind="ExternalInput")
    bvr = nc.dram_tensor("bvr", [128, C], f32, kind="ExternalInput")
    outb = nc.dram_tensor("outb", [128, 1], f32, kind="ExternalInput")
    oneh = nc.dram_tensor("oneh", [128, 8 * 128], f32, kind="ExternalInput")
    ident16 = nc.dram_tensor("ident16", [128, 128], bf16, kind="ExternalInput")
    rotoff = nc.dram_tensor("rotoff", [128, 1], f32, kind="ExternalInput")
    out_d = nc.dram_tensor("out", [C, QS], f32, kind="ExternalOutput")

    with tile.TileContext(nc) as tc, nc.allow_low_precision("bf16 combine by design"):
        with (
            tc.tile_pool(name="const", bufs=1) as cpool,
            tc.tile_pool(name="stage", bufs=1) as spool,
            tc.tile_pool(name="vsb", bufs=3) as vpool,
            tc.tile_pool(name="g", bufs=2) as gpool,
            tc.tile_pool(name="tprime", bufs=2) as tpool,
            tc.tile_pool(name="aggp", bufs=2) as apool,
            tc.tile_pool(name="ps", bufs=1, space="PSUM") as pspool,
            tc.tile_pool(name="dram", bufs=1, space="DRAM") as dpool,
        ):
            # ---- persistent loads ----
            vwT_s = cpool.tile([C, C], bf16)
            nc.sync.dma_start(vwT_s[:], vwT16[:])
            w90T_s = cpool.tile([C, 90], f32)
            nc.sync.dma_start(w90T_s[:], w90T[:])
            owT_s = cpool.tile([C, C], bf16)
            nc.sync.dma_start(owT_s[:], owT16[:])
            b90_s = cpool.tile([128, 90], f32)
            nc.sync.dma_start(b90_s[:], b90r[:])
            bvr_s = cpool.tile([128, C], f32)
            nc.sync.dma_start(bvr_s[:], bvr[:])
            outb_s = cpool.tile([128, 1], f32)
            nc.sync.dma_start(outb_s[:], outb[:])
            oneh_s = cpool.tile([128, 8 * 128], f32)
            nc.sync.dma_start(oneh_s[:], oneh[:])
            ident_s = cpool.tile([128, 128], bf16)
            nc.sync.dma_start(ident_s[:], ident16[:])
            anch_s = cpool.tile([128, NT * 2], f32)
            nc.sync.dma_start(anch_s[:], anch[:])
            rot_s = cpool.tile([128, 1], f32)
            nc.sync.dma_start(rot_s[:], rotoff[:])

            f16s = spool.tile([128, HW], bf16)
            nc.sync.dma_start(f16s[:], feats16[:])
            f32s = spool.tile([128, QS], f32)
            nc.sync.dma_start(f32s[:], feats32[:])

            # pair-row scratch: record r = [V_rot(r), V_rot((r+W) mod HW)].
            # +1 pad record (= record 0) backs the r0+1 read at r0 = HW-1,
            # which is reachable after rotation.
            valscr2 = dpool.tile([HW + 1, 2 * C], bf16)

            proj_s = spool.tile([128, NT * 90], f32)

            # ---- phase A0: query projections (f32, feeds phase B early) ----
            for t in range(NT):
                pps = pspool.tile([128, 90], f32, tag="pp", bufs=2, name=f"pps{t}")
                nc.tensor.matmul(pps[:], f32s[:, t * 128 : (t + 1) * 128],
                                 w90T_s[:], start=True, stop=True)
                nc.vector.tensor_tensor(
                    out=proj_s[:, t * 90 : (t + 1) * 90],
                    in0=pps[:], in1=b90_s[:], op=Alu.add)

            # ---- phase A1: bf16 value map over the whole rotated image ----
            for v in range(NCHV):
                vsb5 = vpool.tile([128, VT * C], bf16, tag="vsb5", name=f"vsb5_{v}")
                for k in range(VT):
                    t = v * VT + k
                    vps = pspool.tile([128, C], f32, tag="mm128", bufs=2,
                                      name=f"vps{t}")
                    nc.tensor.matmul(vps[:], f16s[:, t * 128 : (t + 1) * 128],
                                     vwT_s[:], start=True, stop=True)
                    nc.scalar.copy(vsb5[:, k * C : (k + 1) * C], vps[:])
                base = v * VT * 128  # first pixel (row) of this chunk
                # first half: rows [base, base+640), cols 0:C
                dst1 = AP(tensor=valscr2.tensor,
                          offset=valscr2[:, :].offset + base * 2 * C,
                          ap=[[2 * C, 128], [128 * 2 * C, VT], [1, C]])
                src1 = AP(tensor=vsb5.tensor, offset=vsb5[:, :].offset,
                          ap=[[vsb5[:, :].ap[0][0], 128], [C, VT], [1, C]])
                nc.sync.dma_start(dst1, src1)
                # second half: rows [(base - W) mod HW ...), cols C:2C
                lo = base - W
                if lo >= 0:
                    dst2 = AP(tensor=valscr2.tensor,
                              offset=valscr2[:, :].offset + lo * 2 * C + C,
                              ap=[[2 * C, 128], [128 * 2 * C, VT], [1, C]])
                    nc.sync.dma_start(dst2, src1)
                else:
                    # v == 0: rows [HW-160, HW) from (b=0 all p) + (b=1 p<32),
                    # then rows [0, 96) from (b=1 p>=32), rows [96, 480) b=2..4
                    d_a = AP(tensor=valscr2.tensor,
                             offset=valscr2[:, :].offset + (HW - W) * 2 * C + C,
                             ap=[[2 * C, 128], [1, C]])
                    s_a = AP(tensor=vsb5.tensor, offset=vsb5[:, :].offset,
                             ap=[[vsb5[:, :].ap[0][0], 128], [1, C]])
                    nc.sync.dma_start(d_a, s_a)
                    d_b = AP(tensor=valscr2.tensor,
                             offset=valscr2[:, :].offset + (HW - 32) * 2 * C + C,
                             ap=[[2 * C, 32], [1, C]])
                    s_b = AP(tensor=vsb5.tensor, offset=vsb5[:, :].offset + C,
                             ap=[[vsb5[:, :].ap[0][0], 32], [1, C]])
                    nc.sync.dma_start(d_b, s_b)
                    d_c = AP(tensor=valscr2.tensor,
                             offset=valscr2[:, :].offset + 0 * 2 * C + C,
                             ap=[[2 * C, 96], [1, C]])
                    s_c = AP(tensor=vsb5.tensor,
                             offset=vsb5[32:, :].offset + C,
                             ap=[[vsb5[:, :].ap[0][0], 96], [1, C]])
                    nc.sync.dma_start(d_c, s_c)
                    d_d = AP(tensor=valscr2.tensor,
                             offset=valscr2[:, :].offset + 96 * 2 * C + C,
                             ap=[[2 * C, 128], [128 * 2 * C, VT - 2], [1, C]])
                    s_d = AP(tensor=vsb5.tensor,
                             offset=vsb5[:, :].offset + 2 * C,
                             ap=[[vsb5[:, :].ap[0][0], 128], [C, VT - 2], [1, C]])
                    nc.sync.dma_start(d_d, s_d)
                    # pad record HW = record 0 = [V_rot(0), V_rot(W)]
                    d_p0 = AP(tensor=valscr2.tensor,
                              offset=valscr2[:, :].offset + HW * 2 * C,
                              ap=[[2 * C, 1], [1, C]])
                    s_p0 = AP(tensor=vsb5.tensor, offset=vsb5[:, :].offset,
                              ap=[[vsb5[:, :].ap[0][0], 1], [1, C]])
                    nc.sync.dma_start(d_p0, s_p0)
                    d_p1 = AP(tensor=valscr2.tensor,
                              offset=valscr2[:, :].offset + HW * 2 * C + C,
                              ap=[[2 * C, 1], [1, C]])
                    s_p1 = AP(tensor=vsb5.tensor,
                              offset=vsb5[32:, :].offset + C,
                              ap=[[vsb5[:, :].ap[0][0], 1], [1, C]])
                    nc.sync.dma_start(d_p1, s_p1)

            # ---- phase B: softmax / coords / weights (query-major) ----
            # proj_s free layout per tile t: [0,72) = wlog (pt*8+g), [72,90) = offs
            def tt(out, in0, in1, op):
                nc.vector.tensor_tensor(out=out, in0=in0, in1=in1, op=op)

            # softmax over points
            wmax = spool.tile([128, NT * G], f32)
            wl_gp = AP(tensor=proj_s.tensor, offset=proj_s[:, :].offset,
                       ap=[[proj_s[:, :].ap[0][0], 128], [90, NT], [1, G], [G, P]])
            nc.vector.tensor_reduce(out=wmax[:, :].rearrange("p (t g) -> p t g", g=G),
                                    in_=wl_gp, axis=Ax.X, op=Alu.max)
            smf = spool.tile([128, NT * P * G], f32, tag="smf")
            wl_pg = AP(tensor=proj_s.tensor, offset=proj_s[:, :].offset,
                       ap=[[proj_s[:, :].ap[0][0], 128], [90, NT], [G, P], [1, G]])
            wmax_b = AP(tensor=wmax.tensor, offset=wmax[:, :].offset,
                        ap=[[wmax[:, :].ap[0][0], 128], [G, NT], [0, P], [1, G]])
            tt(smf[:, :].rearrange("p (t q g) -> p t q g", q=P, g=G),
               wl_pg, wmax_b, Alu.subtract)
            nc.scalar.activation(smf[:], smf[:], Act.Exp)
            ssum = spool.tile([128, NT * G], f32)
            sm_gp = AP(tensor=smf.tensor, offset=smf[:, :].offset,
                       ap=[[smf[:, :].ap[0][0], 128], [P * G, NT], [1, G], [G, P]])
            nc.vector.tensor_reduce(out=ssum[:, :].rearrange("p (t g) -> p t g", g=G),
                                    in_=sm_gp, axis=Ax.X, op=Alu.add)
            rcps = spool.tile([128, NT * G], f32)
            nc.vector.reciprocal(rcps[:], ssum[:])
            wsm = spool.tile([128, NT * P * G], bf16)
            rcp_b = AP(tensor=rcps.tensor, offset=rcps[:, :].offset,
                       ap=[[rcps[:, :].ap[0][0], 128], [G, NT], [0, P], [1, G]])
            tt(wsm[:, :].rearrange("p (t q g) -> p t q g", q=P, g=G),
               smf[:, :].rearrange("p (t q g) -> p t q g", q=P, g=G),
               rcp_b, Alu.mult)

            # coords: px/py [128, NT*P] laid out (t, pt)
            px = spool.tile([128, NP_], f32, tag="px")
            py = spool.tile([128, NP_], f32, tag="py")
            offs_x = AP(tensor=proj_s.tensor, offset=proj_s[:, :].offset + 72,
                        ap=[[proj_s[:, :].ap[0][0], 128], [90, NT], [2, P]])
            offs_y = AP(tensor=proj_s.tensor, offset=proj_s[:, :].offset + 73,
                        ap=[[proj_s[:, :].ap[0][0], 128], [90, NT], [2, P]])
            anx = AP(tensor=anch_s.tensor, offset=anch_s[:, :].offset,
                     ap=[[anch_s[:, :].ap[0][0], 128], [2, NT], [0, P]])
            any_ = AP(tensor=anch_s.tensor, offset=anch_s[:, :].offset + 1,
                      ap=[[anch_s[:, :].ap[0][0], 128], [2, NT], [0, P]])
            tt(px[:, :].rearrange("p (t q) -> p t q", q=P), offs_x, anx, Alu.add)
            tt(py[:, :].rearrange("p (t q) -> p t q", q=P), offs_y, any_, Alu.add)

            xp = spool.tile([128, NP_], f32)
            yp = spool.tile([128, NP_], f32)
            nc.scalar.activation(xp[:], px[:], Act.Copy, bias=SHIFT - 0.5, scale=float(W))
            nc.scalar.activation(yp[:], py[:], Act.Copy, bias=SHIFT - 0.5, scale=float(H))
            # floor via round(x-0.5): (x + (2^23-0.5)) - 2^23. At integer x the
            # half-even tie may floor one low with frac 1.0 — an equivalent
            # bilinear weighting, so interpolation is unchanged.
            MAGIC = float(1 << 23)
            xf = spool.tile([128, NP_], f32, tag="px")   # reuse px slot
            yf = spool.tile([128, NP_], f32, tag="py")   # reuse py slot
            nc.vector.tensor_scalar(out=xf[:], in0=xp[:], scalar1=MAGIC - 0.5,
                                    scalar2=MAGIC, op0=Alu.add, op1=Alu.subtract)
            nc.vector.tensor_scalar(out=yf[:], in0=yp[:], scalar1=MAGIC - 0.5,
                                    scalar2=MAGIC, op0=Alu.add, op1=Alu.subtract)
            wx = spool.tile([128, NP_], f32)
            wy = spool.tile([128, NP_], f32)
            tt(wx[:], xp[:], xf[:], Alu.subtract)
            tt(wy[:], yp[:], yf[:], Alu.subtract)

            xg = spool.tile([128, NP_], f32)
            nc.vector.tensor_scalar(out=xg[:], in0=xf[:], scalar1=SHIFT, scalar2=0.0,
                                    op0=Alu.subtract, op1=Alu.max)
            nc.vector.tensor_scalar(out=xg[:], in0=xg[:], scalar1=float(W - 2),
                                    scalar2=None, op0=Alu.min)
            yg = spool.tile([128, NP_], f32)
            nc.vector.tensor_scalar(out=yg[:], in0=yf[:], scalar1=SHIFT, scalar2=0.0,
                                    op0=Alu.subtract, op1=Alu.max)
            nc.vector.tensor_scalar(out=yg[:], in0=yg[:], scalar1=float(H - 2),
                                    scalar2=None, op0=Alu.min)

            ux = spool.tile([128, NP_], f32)
            uy = spool.tile([128, NP_], f32)
            nc.scalar.activation(ux[:], wx[:], Act.Copy, bias=1.0, scale=-1.0)
            nc.scalar.activation(uy[:], wy[:], Act.Copy, bias=1.0, scale=-1.0)

            # x-validity masks (with pair-clamp weight swap)
            tA = spool.tile([128, NP_], f32)
            tB = spool.tile([128, NP_], f32)
            mAx = spool.tile([128, NP_], f32)
            nc.vector.tensor_scalar(out=tA[:], in0=xf[:], scalar1=SHIFT,
                                    scalar2=None, op0=Alu.is_ge)
            nc.vector.tensor_scalar(out=tB[:], in0=xf[:], scalar1=SHIFT + W - 2,
                                    scalar2=None, op0=Alu.is_le)
            tt(mAx[:], tA[:], tB[:], Alu.mult)
            mBx = spool.tile([128, NP_], f32)
            nc.vector.tensor_scalar(out=mBx[:], in0=xf[:], scalar1=SHIFT - 1.0,
                                    scalar2=None, op0=Alu.is_equal)
            mCx = spool.tile([128, NP_], f32)
            nc.vector.tensor_scalar(out=mCx[:], in0=xf[:], scalar1=SHIFT + W - 1,
                                    scalar2=None, op0=Alu.is_equal)

            bx = spool.tile([128, NT * P * 2], f32)   # (t, pt, side)
            v1 = spool.tile([128, NP_], f32)
            v2 = spool.tile([128, NP_], f32)
            tt(v1[:], ux[:], mAx[:], Alu.mult)
            tt(v2[:], wx[:], mBx[:], Alu.mult)
            bx0 = AP(tensor=bx.tensor, offset=bx[:, :].offset,
                     ap=[[bx[:, :].ap[0][0], 128], [2, NP_]])
            tt(bx0, v1[:], v2[:], Alu.add)
            tt(v1[:], wx[:], mAx[:], Alu.mult)
            tt(v2[:], ux[:], mCx[:], Alu.mult)
            bx1 = AP(tensor=bx.tensor, offset=bx[:, :].offset + 1,
                     ap=[[bx[:, :].ap[0][0], 128], [2, NP_]])
            tt(bx1, v1[:], v2[:], Alu.add)

            # y masks (swap form, mirrors x): slot0 = row yg, slot1 = row yg+1
            my = spool.tile([128, NP_], f32)
            nc.vector.tensor_scalar(out=tA[:], in0=yf[:], scalar1=SHIFT,
                                    scalar2=None, op0=Alu.is_ge)
            nc.vector.tensor_scalar(out=tB[:], in0=yf[:], scalar1=SHIFT + H - 2,
                                    scalar2=None, op0=Alu.is_le)
            tt(my[:], tA[:], tB[:], Alu.mult)
            myB = spool.tile([128, NP_], f32)
            nc.vector.tensor_scalar(out=myB[:], in0=yf[:], scalar1=SHIFT - 1.0,
                                    scalar2=None, op0=Alu.is_equal)
            myC = spool.tile([128, NP_], f32)
            nc.vector.tensor_scalar(out=myC[:], in0=yf[:], scalar1=SHIFT + H - 1,
                                    scalar2=None, op0=Alu.is_equal)
            by = spool.tile([128, NT * P * 2], f32)   # (t, pt, yy)
            by0 = AP(tensor=by.tensor, offset=by[:, :].offset,
                     ap=[[by[:, :].ap[0][0], 128], [2, NP_]])
            by1 = AP(tensor=by.tensor, offset=by[:, :].offset + 1,
                     ap=[[by[:, :].ap[0][0], 128], [2, NP_]])
            tt(v1[:], uy[:], my[:], Alu.mult)
            tt(v2[:], wy[:], myB[:], Alu.mult)
            tt(by0, v1[:], v2[:], Alu.add)
            tt(v1[:], wy[:], my[:], Alu.mult)
            tt(v2[:], uy[:], myC[:], Alu.mult)
            tt(by1, v1[:], v2[:], Alu.add)

            # gather record indices (rotated): idx = (yg*W + xg - rotoff) mod HW
            idxf = spool.tile([128, NP_], f32)
            nc.scalar.activation(v1[:], yg[:], Act.Copy, bias=0.0, scale=float(W))
            tt(idxf[:], v1[:], xg[:], Alu.add)
            nc.vector.tensor_scalar(out=idxf[:], in0=idxf[:], scalar1=rot_s[:, 0:1],
                                    scalar2=None, op0=Alu.subtract)
            wrap = spool.tile([128, NP_], f32, tag="v1b")
            nc.vector.tensor_scalar(out=wrap[:], in0=idxf[:], scalar1=0.0,
                                    scalar2=float(HW), op0=Alu.is_lt, op1=Alu.mult)
            tt(idxf[:], idxf[:], wrap[:], Alu.add)

            # cw[t, pt, x, yy] = bx[t,pt,x] * by[t,pt,yy]
            cw = spool.tile([128, NT * P * 4], f32)
            for x in range(2):
                by_v = AP(tensor=by.tensor, offset=by[:, :].offset,
                          ap=[[by[:, :].ap[0][0], 128], [2 * P, NT], [2, P], [1, 2]])
                bx_x = AP(tensor=bx.tensor, offset=bx[:, :].offset + x,
                          ap=[[bx[:, :].ap[0][0], 128], [2 * P, NT], [2, P], [0, 2]])
                cw_x = AP(tensor=cw.tensor, offset=cw[:, :].offset + 2 * x,
                          ap=[[cw[:, :].ap[0][0], 128], [4 * P, NT], [4, P], [1, 2]])
                tt(cw_x, by_v, bx_x, Alu.mult)

            # kw[t, pt, rs, g] = cw[t, pt, rs] * wsm[t, pt, g]  (bf16)
            kw = spool.tile([128, NT * P * 4 * G], bf16)
            for rs in range(4):
                cw_rs = AP(tensor=cw.tensor, offset=cw[:, :].offset + rs,
                           ap=[[cw[:, :].ap[0][0], 128], [4 * P, NT], [4, P], [0, G]])
                w_v = AP(tensor=wsm.tensor, offset=wsm[:, :].offset,
                         ap=[[wsm[:, :].ap[0][0], 128], [P * G, NT], [G, P], [1, G]])
                kw_rs = AP(tensor=kw.tensor, offset=kw[:, :].offset + rs * G,
                           ap=[[kw[:, :].ap[0][0], 128], [4 * P * G, NT], [4 * G, P], [1, G]])
                tt(kw_rs, cw_rs, w_v, Alu.mult)

            # sumcoef[t, g] = sum_pt wsm * (bx0+bx1)*(by0+by1)   (for value_b)
            bx0r = AP(tensor=bx.tensor, offset=bx[:, :].offset,
                      ap=[[bx[:, :].ap[0][0], 128], [2, NP_]])
            bx1r = AP(tensor=bx.tensor, offset=bx[:, :].offset + 1,
                      ap=[[bx[:, :].ap[0][0], 128], [2, NP_]])
            by0r = AP(tensor=by.tensor, offset=by[:, :].offset,
                      ap=[[by[:, :].ap[0][0], 128], [2, NP_]])
            by1r = AP(tensor=by.tensor, offset=by[:, :].offset + 1,
                      ap=[[by[:, :].ap[0][0], 128], [2, NP_]])
            tt(v1[:], bx0r, bx1r, Alu.add)
            tt(v2[:], by0r, by1r, Alu.add)
            bws = spool.tile([128, NP_], bf16)
            tt(bws[:], v1[:], v2[:], Alu.mult)
            wp = spool.tile([128, NT * P * G], bf16, tag="smf")  # reuse smf slot
            bws_b = AP(tensor=bws.tensor, offset=bws[:, :].offset,
                       ap=[[bws[:, :].ap[0][0], 128], [P, NT], [1, P], [0, G]])
            tt(wp[:, :].rearrange("p (t q g) -> p t q g", q=P, g=G),
               wsm[:, :].rearrange("p (t q g) -> p t q g", q=P, g=G),
               bws_b, Alu.mult)
            sumcoef = spool.tile([128, NT * G], f32)
            wp_gp = AP(tensor=wp.tensor, offset=wp[:, :].offset,
                       ap=[[wp[:, :].ap[0][0], 128], [P * G, NT], [1, G], [G, P]])
            nc.vector.tensor_reduce(out=sumcoef[:, :].rearrange("p (t g) -> p t g", g=G),
                                    in_=wp_gp, axis=Ax.X, op=Alu.add)

            # ---- idx16: 8 one-hot matmuls over all points, strided copies ----
            # position i in chunk ch = jj*128 + q (jj = tt*P + pt), stored at
            # idx16[q%16 (+16a), ch*144 + 8*jj + q//16]
            idx16 = spool.tile([128, NCH * (NIDX_CH // 16)], i16)
            for qh in range(8):
                i16ps = pspool.tile([128, NP_], f32, tag="pp", bufs=2,
                                    name=f"i16ps{qh}")
                nc.tensor.matmul(i16ps[:], oneh_s[:, qh * 128 : (qh + 1) * 128],
                                 idxf[:], start=True, stop=True)
                dst = AP(tensor=idx16.tensor, offset=idx16[:, :].offset + qh,
                         ap=[[idx16[:, :].ap[0][0], 128],
                             [NIDX_CH // 16, NCH], [8 * P, TCH], [8, P]])
                src = AP(tensor=i16ps.tensor, offset=i16ps[:, :].offset,
                         ap=[[i16ps[:, :].ap[0][0], 128],
                             [TCH * P, NCH], [P, TCH], [1, P]])
                nc.vector.tensor_copy(dst, src)

            # ---- phase C: gather chunks + combine ----
            val_src = AP(tensor=valscr2.tensor, offset=valscr2[:, :].offset,
                         ap=[[2 * C, HW], [1, 4 * C]])

            for ch in range(NCH):
                gt = gpool.tile([128, TCH * P, 4 * C], bf16, tag="g", name=f"g{ch}")
                nc.gpsimd.dma_gather(
                    gt[:, :, :], val_src,
                    idx16[:, ch * (NIDX_CH // 16) : (ch + 1) * (NIDX_CH // 16)],
                    num_idxs=NIDX_CH, num_idxs_reg=NIDX_CH,
                    elem_size=4 * C, elem_step=2 * C, single_packet=False,
                )

                for tt_ in range(TCH):
                    t = ch * TCH + tt_
                    # tp[q, (j36, c)] = gt * kw  (j36 = (pt, x, yy), c = (g, gc))
                    tp = tpool.tile([128, P * 4 * C], bf16, tag="tp", name=f"tp{t}")
                    g_v = AP(tensor=gt.tensor,
                             offset=gt[:, :, :].offset + tt_ * P * 4 * C,
                             ap=[[gt[:, :, :].ap[0][0], 128],
                                 [C, P * 4], [GC, G], [1, GC]])
                    kw_v = AP(tensor=kw.tensor, offset=kw[:, :].offset + t * P * 4 * G,
                              ap=[[kw[:, :].ap[0][0], 128],
                                  [G, P * 4], [1, G], [0, GC]])
                    tp_v = AP(tensor=tp.tensor, offset=tp[:, :].offset,
                              ap=[[tp[:, :].ap[0][0], 128],
                                  [C, P * 4], [GC, G], [1, GC]])
                    nc.vector.tensor_tensor(out=tp_v, in0=g_v, in1=kw_v, op=Alu.mult)

                    # pre-add yy pairs: tp2[q, (pt, x, c)] (2x-mode, step-1)
                    tp2 = tpool.tile([128, P * 2 * C], bf16, tag="tp2", name=f"tp2{t}")
                    in0 = AP(tensor=tp.tensor, offset=tp[:, :].offset,
                             ap=[[tp[:, :].ap[0][0], 128], [2 * C, 2 * P], [1, C]])
                    in1 = AP(tensor=tp.tensor, offset=tp[:, :].offset + C,
                             ap=[[tp[:, :].ap[0][0], 128], [2 * C, 2 * P], [1, C]])
                    out2 = AP(tensor=tp2.tensor, offset=tp2[:, :].offset,
                              ap=[[tp2[:, :].ap[0][0], 128], [C, 2 * P], [1, C]])
                    nc.vector.tensor_tensor(out=out2, in0=in0, in1=in1, op=Alu.add)

                    # ebias[q, c] = value_b[c] * sumcoef[q, g(c)]  (bf16)
                    ebias = apool.tile([128, C], bf16, tag="eb", name=f"eb{t}")
                    sc_v = AP(tensor=sumcoef.tensor,
                              offset=sumcoef[:, :].offset + t * G,
                              ap=[[sumcoef[:, :].ap[0][0], 128], [1, G], [0, GC]])
                    bv_v = bvr_s[:, :].rearrange("p (g c) -> p g c", g=G)
                    nc.vector.tensor_tensor(
                        out=ebias[:, :].rearrange("p (g c) -> p g c", g=G),
                        in0=sc_v, in1=bv_v, op=Alu.mult)

                    # 19 transpose-accumulate matmuls: aggT[c, q] in PSUM
                    aggT = pspool.tile([128, 128], f32, tag="mm128", bufs=2,
                                       name=f"aggT{t}")
                    for k in range(2 * P):
                        nc.tensor.matmul(aggT[:], tp2[:, k * C : (k + 1) * C],
                                         ident_s[:], start=(k == 0), stop=False)
                    nc.tensor.matmul(aggT[:], ebias[:], ident_s[:],
                                     start=False, stop=True)

                    # out-projection straight off the accumulated PSUM
                    aT = apool.tile([128, 128], bf16, tag="aT", name=f"aT{t}")
                    nc.scalar.copy(aT[:], aggT[:])
                    fops = pspool.tile([128, C], f32, tag="fo", bufs=2,
                                       name=f"fo{t}")
                    nc.tensor.matmul(fops[:], owT_s[:], aT[:], start=True, stop=True)
                    fo_sb = apool.tile([128, C], f32, tag="fosb", name=f"fosb{t}")
                    nc.scalar.activation(fo_sb[:], fops[:], Act.Identity,
                                         bias=outb_s[:, 0:1], scale=1.0)
                    nc.sync.dma_start(out_d[:, t * 128 : (t + 1) * 128], fo_sb[:])

    nc.finalize()
    return nc


def _host_prep(inputs):
    """Prepare per-core input maps from full inputs."""
    feats = np.asarray(inputs["feats"], np.float32)          # [B, C, H, W]
    anchor = np.asarray(inputs["anchor_points"], np.float32)  # [B, HW, 2]
    value_w = np.asarray(inputs["value_w"], np.float32)
    value_b = np.asarray(inputs["value_b"], np.float32)
    weights_w = np.asarray(inputs["weights_w"], np.float32)
    weights_b = np.asarray(inputs["weights_b"], np.float32)
    offset_w = np.asarray(inputs["offset_w"], np.float32)
    offset_b = np.asarray(inputs["offset_b"], np.float32)
    out_w = np.asarray(inputs["out_w"], np.float32)
    out_b = np.asarray(inputs["out_b"], np.float32)

    w90 = np.concatenate([weights_w, offset_w], 0)            # [90, C]
    b90 = np.concatenate([weights_b, offset_b], 0)            # [90]
    shared = {
        "vwT16": np.ascontiguousarray(value_w.T).astype(ml_dtypes.bfloat16),
        "w90T": np.ascontiguousarray(w90.T),
        "owT16": np.ascontiguousarray(out_w.T).astype(ml_dtypes.bfloat16),
        "b90r": np.broadcast_to(b90, (128, 90)).copy(),
        "bvr": np.broadcast_to(value_b, (128, C)).copy(),
        "outb": out_b.reshape(128, 1).copy(),
        "ident16": np.eye(128, dtype=ml_dtypes.bfloat16),
    }
    oneh = np.zeros((128, 8, 128), np.float32)
    for qh in range(8):
        for m in range(128):
            oneh[16 * qh + (m % 16), qh, m] = 1.0
    shared["oneh"] = oneh.reshape(128, 8 * 128)

    in_maps = []
    feats16_b = [feats[b].reshape(C, HW).astype(ml_dtypes.bfloat16) for b in range(B)]
    for core in range(NCORES):
        b_i, sl = core // 4, core % 4
        off = sl * QS
        f16 = np.roll(feats16_b[b_i], -off, axis=1)
        f32r = np.roll(feats[b_i].reshape(C, HW), -off, axis=1)[:, :QS]
        an = anchor[b_i, off : off + QS].reshape(NT, 128, 2).transpose(1, 0, 2).reshape(128, NT * 2)
        m = dict(shared)
        m["feats16"] = np.ascontiguousarray(f16)
        m["feats32"] = np.ascontiguousarray(f32r)
        m["anch"] = np.ascontiguousarray(an)
        m["rotoff"] = np.full((128, 1), float(off), np.float32)
        in_maps.append(m)
    return in_maps


def kernel(**inputs) -> np.ndarray:
    from concourse.bass_utils import run_bass_kernel_spmd

    if "nc" not in _CACHE:
        _CACHE["nc"] = _build_nc()
    nc = _CACHE["nc"]
    in_maps = _host_prep(inputs)
    res = run_bass_kernel_spmd(nc, in_maps, core_ids=list(range(NCORES)))
    out = np.zeros((B, C, HW), np.float32)
    for core in range(NCORES):
        b_i, sl = core // 4, core % 4
        out[b_i, :, sl * QS : (sl + 1) * QS] = res.results[core]["out"]
    return out.reshape(B, C, H, W)


# revision 12
# speedup vs baseline: 1.8585x; 1.0202x over previous
"""Deformable 2D feature aggregator — Trainium2 Bass kernel, 8-core SPMD. v3.

Problem: B=2, C=128, H=96, W=160, P=9 points, G=8 groups.
  value = conv1x1(feats); w = softmax over P of conv1x1(feats); offs = conv1x1(feats)
  pts = anchors + offs; out_proj(conv-weighted bilinear gather of value at pts).

Sharding: 8 cores = 2 batches x 4 query-slices; each core builds the full
(rotated) value map for its batch, then gathers per-query corner windows.

v3 vs v2:
  - coords -> idxf -> idx16 chain emitted FIRST so the gather stream (gated on
    idx16 + valscr2) starts ~50us earlier; softmax/kw overlap under gathers.
  - last two gather chunks are single-tile so the final combine tail is half.
  - gather pool triple-buffered to absorb combine latency between chunks.
"""
import sys

sys.path.insert(0, "/opt/trn_rl_repo")

import numpy as np
import ml_dtypes

import concourse.bass as bass
import concourse.bacc as bacc
import concourse.mybir as mybir
import concourse.tile as tile
from concourse import library_config
from concourse.ap import AP

# problem constants (hardcoded per harness contract)
B, C, H, W = 2, 128, 96, 160
HW = H * W                     # 15360
P, G, GC = 9, 8, 16
NCORES = 8
QS = B * HW // NCORES          # 3840 queries per core
NT = QS // 128                 # 30 query tiles
TCH = 2                        # query tiles per gather chunk
NCH = NT // TCH                # 15 gather chunks
NP_ = NT * P                   # 270 points per partition-row
NIDX_CH = TCH * 128 * P        # 2304 gather indices per chunk
SHIFT = 1024.0                 # floor-bias (exact in f32 for our range)
NPXT = HW // 128               # 120 pixel tiles
VT = 10                        # pixel tiles per valscr2 write chunk
NCHV = NPXT // VT              # 24 write chunks

f32 = mybir.dt.float32
bf16 = mybir.dt.bfloat16
i16 = mybir.dt.int16
Alu = mybir.AluOpType
Act = mybir.ActivationFunctionType
Ax = mybir.AxisListType

_CACHE: dict = {}


def _build_nc():
    nc = bacc.Bacc()

    feats16 = nc.dram_tensor("feats16", [C, HW], bf16, kind="ExternalInput")
    feats32 = nc.dram_tensor("feats32", [C, QS], f32, kind="ExternalInput")
    anch = nc.dram_tensor("anch", [128, NT * 2], f32, kind="ExternalInput")
    vwT16 = nc.dram_tensor("vwT16", [C, C], bf16, kind="ExternalInput")
    w72T = nc.dram_tensor("w72T", [C, 72], bf16, kind="ExternalInput")
    w18T = nc.dram_tensor("w18T", [C, 18], f32, kind="ExternalInput")
    owT16 = nc.dram_tensor("owT16", [C, C], bf16, kind="ExternalInput")
    b72r = nc.dram_tensor("b72r", [128, 72], f32, kind="ExternalInput")
    b18r = nc.dram_tensor("b18r", [128, 18], f32, kind="ExternalInput")
    bvr = nc.dram_tensor("bvr", [128, C], f32, kind="ExternalInput")
    outb = nc.dram_tensor("outb", [128, 1], f32, kind="ExternalInput")
    oneh = nc.dram_tensor("oneh", [128, 8 * 128], f32, kind="ExternalInput")
    ident16 = nc.dram_tensor("ident16", [128, 128], bf16, kind="ExternalInput")
    rotoff = nc.dram_tensor("rotoff", [128, 1], f32, kind="ExternalInput")
    out_d = nc.dram_tensor("out", [C, QS], f32, kind="ExternalOutput")

    with tile.TileContext(nc) as tc, nc.allow_low_precision("bf16 combine by design"):
        with (
            tc.tile_pool(name="const", bufs=1) as cpool,
            tc.tile_pool(name="stage", bufs=1) as spool,
            tc.tile_pool(name="vsb", bufs=3) as vpool,
            tc.tile_pool(name="g", bufs=2) as gpool,
            tc.tile_pool(name="tprime", bufs=2) as tpool,
            tc.tile_pool(name="aggp", bufs=2) as apool,
            tc.tile_pool(name="ps", bufs=1, space="PSUM") as pspool,
            tc.tile_pool(name="dram", bufs=1, space="DRAM") as dpool,
        ):
            # ---- persistent loads (query-path tensors first: they gate idx16) ----
            f32s = spool.tile([128, QS], f32)
            nc.sync.dma_start(f32s[:], feats32[:])
            w72T_s = cpool.tile([C, 72], bf16)
            nc.sync.dma_start(w72T_s[:], w72T[:])
            w18T_s = cpool.tile([C, 18], f32)
            nc.sync.dma_start(w18T_s[:], w18T[:])
            anch_s = cpool.tile([128, NT * 2], f32)
            nc.sync.dma_start(anch_s[:], anch[:])
            rot_s = cpool.tile([128, 1], f32)
            nc.sync.dma_start(rot_s[:], rotoff[:])
            oneh_s = cpool.tile([128, 8 * 128], f32)
            nc.sync.dma_start(oneh_s[:], oneh[:])
            b72_s = cpool.tile([128, 72], f32)
            nc.sync.dma_start(b72_s[:], b72r[:])
            b18_s = cpool.tile([128, 18], f32)
            nc.sync.dma_start(b18_s[:], b18r[:])
            f16s = spool.tile([128, HW], bf16)
            nc.sync.dma_start(f16s[:], feats16[:])
            vwT_s = cpool.tile([C, C], bf16)
            nc.sync.dma_start(vwT_s[:], vwT16[:])
            owT_s = cpool.tile([C, C], bf16)
            nc.sync.dma_start(owT_s[:], owT16[:])
            bvr_s = cpool.tile([128, C], f32)
            nc.sync.dma_start(bvr_s[:], bvr[:])
            outb_s = cpool.tile([128, 1], f32)
            nc.sync.dma_start(outb_s[:], outb[:])
            ident_s = cpool.tile([128, 128], bf16)
            nc.sync.dma_start(ident_s[:], ident16[:])

            # pair-row scratch: record r = [V_rot(r), V_rot((r+W) mod HW)].
            # +1 pad record (= record 0) backs the r0+1 read at r0 = HW-1,
            # which is reachable after rotation.
            valscr2 = dpool.tile([HW + 1, 2 * C], bf16)

            proj72 = spool.tile([128, NT * 72], bf16)
            proj18 = spool.tile([128, NT * 18], f32)

            def tt(out, in0, in1, op):
                nc.vector.tensor_tensor(out=out, in0=in0, in1=in1, op=op)

            # ---- phase A0: query projections (offsets f32; wlog bf16) ----
            for t in range(NT):
                pp18 = pspool.tile([128, 18], f32, tag="pp", bufs=2,
                                   name=f"pp18_{t}")
                nc.tensor.matmul(pp18[:], f32s[:, t * 128 : (t + 1) * 128],
                                 w18T_s[:], start=True, stop=True)
                nc.vector.tensor_tensor(
                    out=proj18[:, t * 18 : (t + 1) * 18],
                    in0=pp18[:], in1=b18_s[:], op=Alu.add)
            for t in range(NT):
                pp72 = pspool.tile([128, 72], f32, tag="pp", bufs=2,
                                   name=f"pp72_{t}")
                nc.tensor.matmul(pp72[:], f16s[:, t * 128 : (t + 1) * 128],
                                 w72T_s[:], start=True, stop=True)
                nc.vector.tensor_tensor(
                    out=proj72[:, t * 72 : (t + 1) * 72],
                    in0=pp72[:], in1=b72_s[:], op=Alu.add)

            # ---- phase B0: coords -> gather record indices (critical path) ----
            px = spool.tile([128, NP_], f32, tag="px")
            py = spool.tile([128, NP_], f32, tag="py")
            offs_x = AP(tensor=proj18.tensor, offset=proj18[:, :].offset,
                        ap=[[proj18[:, :].ap[0][0], 128], [18, NT], [2, P]])
            offs_y = AP(tensor=proj18.tensor, offset=proj18[:, :].offset + 1,
                        ap=[[proj18[:, :].ap[0][0], 128], [18, NT], [2, P]])
            anx = AP(tensor=anch_s.tensor, offset=anch_s[:, :].offset,
                     ap=[[anch_s[:, :].ap[0][0], 128], [2, NT], [0, P]])
            any_ = AP(tensor=anch_s.tensor, offset=anch_s[:, :].offset + 1,
                      ap=[[anch_s[:, :].ap[0][0], 128], [2, NT], [0, P]])
            tt(px[:, :].rearrange("p (t q) -> p t q", q=P), offs_x, anx, Alu.add)
            tt(py[:, :].rearrange("p (t q) -> p t q", q=P), offs_y, any_, Alu.add)

            xp = spool.tile([128, NP_], f32)
            yp = spool.tile([128, NP_], f32)
            nc.scalar.activation(xp[:], px[:], Act.Copy, bias=SHIFT - 0.5, scale=float(W))
            nc.scalar.activation(yp[:], py[:], Act.Copy, bias=SHIFT - 0.5, scale=float(H))
            # floor via round(x-0.5): (x + (2^23-0.5)) - 2^23. At integer x the
            # half-even tie may floor one low with frac 1.0 — an equivalent
            # bilinear weighting, so interpolation is unchanged.
            MAGIC = float(1 << 23)
            xf = spool.tile([128, NP_], f32, tag="px")   # reuse px slot
            yf = spool.tile([128, NP_], f32, tag="py")   # reuse py slot
            nc.vector.tensor_scalar(out=xf[:], in0=xp[:], scalar1=MAGIC - 0.5,
                                    scalar2=MAGIC, op0=Alu.add, op1=Alu.subtract)
            nc.vector.tensor_scalar(out=yf[:], in0=yp[:], scalar1=MAGIC - 0.5,
                                    scalar2=MAGIC, op0=Alu.add, op1=Alu.subtract)

            xg = spool.tile([128, NP_], f32)
            nc.vector.tensor_scalar(out=xg[:], in0=xf[:], scalar1=SHIFT, scalar2=0.0,
                                    op0=Alu.subtract, op1=Alu.max)
            nc.vector.tensor_scalar(out=xg[:], in0=xg[:], scalar1=float(W - 2),
                                    scalar2=None, op0=Alu.min)
            yg = spool.tile([128, NP_], f32)
            nc.vector.tensor_scalar(out=yg[:], in0=yf[:], scalar1=SHIFT, scalar2=0.0,
                                    op0=Alu.subtract, op1=Alu.max)
            nc.vector.tensor_scalar(out=yg[:], in0=yg[:], scalar1=float(H - 2),
                                    scalar2=None, op0=Alu.min)

            # idx = (yg*W + xg - rotoff) mod HW
            v1 = spool.tile([128, NP_], f32)
            v2 = spool.tile([128, NP_], f32)
            idxf = spool.tile([128, NP_], f32)
            nc.scalar.activation(v1[:], yg[:], Act.Copy, bias=0.0, scale=float(W))
            tt(idxf[:], v1[:], xg[:], Alu.add)
            nc.vector.tensor_scalar(out=idxf[:], in0=idxf[:], scalar1=rot_s[:, 0:1],
                                    scalar2=None, op0=Alu.subtract)
            nc.vector.tensor_scalar(out=v2[:], in0=idxf[:], scalar1=0.0,
                                    scalar2=float(HW), op0=Alu.is_lt, op1=Alu.mult)
            tt(idxf[:], idxf[:], v2[:], Alu.add)

            # chunk schedule: 14 double-tile chunks + 2 single-tile chunks
            # (small tail chunks shorten the final gather-drain + combine).
            CHUNKS = [(2 * i, 2) for i in range(14)] + [(28, 1), (29, 1)]
            COLOFF = []
            acc = 0
            for (_, ntile) in CHUNKS:
                COLOFF.append(acc)
                acc += ntile * P * 8

            # idx16: position i in chunk = jj*128 + q (jj = tt*P + pt),
            # stored at idx16[q%16 (+16a), coloff + 8*jj + q//16]
            idx16 = spool.tile([128, NT * P * 8], i16)
            for qh in range(8):
                i16ps = pspool.tile([128, NP_], f32, tag="pp", bufs=2,
                                    name=f"i16ps{qh}")
                nc.tensor.matmul(i16ps[:], oneh_s[:, qh * 128 : (qh + 1) * 128],
                                 idxf[:], start=True, stop=True)
                dst = AP(tensor=idx16.tensor, offset=idx16[:, :].offset + qh,
                         ap=[[idx16[:, :].ap[0][0], 128],
                             [2 * P * 8, 14], [P * 8, 2], [8, P]])
                src = AP(tensor=i16ps.tensor, offset=i16ps[:, :].offset,
                         ap=[[i16ps[:, :].ap[0][0], 128],
                             [2 * P, 14], [P, 2], [1, P]])
                nc.vector.tensor_copy(dst, src)
                # tail tiles 28, 29 (single-tile chunks, contiguous blocks)
                dst_t = AP(tensor=idx16.tensor,
                           offset=idx16[:, :].offset + qh + 28 * P * 8,
                           ap=[[idx16[:, :].ap[0][0], 128], [P * 8, 2], [8, P]])
                src_t = AP(tensor=i16ps.tensor,
                           offset=i16ps[:, :].offset + 28 * P,
                           ap=[[i16ps[:, :].ap[0][0], 128], [P, 2], [1, P]])
                nc.vector.tensor_copy(dst_t, src_t)

            # ---- phase A1: bf16 value map over the whole rotated image ----
            for v in range(NCHV):
                vsb5 = vpool.tile([128, VT * C], bf16, tag="vsb5", bufs=2,
                                  name=f"vsb5_{v}")
                for k2 in range(VT // 2):
                    t = v * VT + 2 * k2
                    vps = pspool.tile([128, 2 * C], f32, tag="mm128", bufs=2,
                                      name=f"vps{t}")
                    nc.tensor.matmul(vps[:, 0:C], f16s[:, t * 128 : (t + 1) * 128],
                                     vwT_s[:], start=True, stop=True)
                    nc.tensor.matmul(vps[:, C : 2 * C],
                                     f16s[:, (t + 1) * 128 : (t + 2) * 128],
                                     vwT_s[:], start=True, stop=True)
                    nc.scalar.copy(vsb5[:, 2 * k2 * C : (2 * k2 + 2) * C], vps[:])
                base = v * VT * 128  # first pixel (row) of this chunk
                # first half: rows [base, base+640), cols 0:C
                dst1 = AP(tensor=valscr2.tensor,
                          offset=valscr2[:, :].offset + base * 2 * C,
                          ap=[[2 * C, 128], [128 * 2 * C, VT], [1, C]])
                src1 = AP(tensor=vsb5.tensor, offset=vsb5[:, :].offset,
                          ap=[[vsb5[:, :].ap[0][0], 128], [C, VT], [1, C]])
                nc.sync.dma_start(dst1, src1)
                # second half: rows [(base - W) mod HW ...), cols C:2C
                lo = base - W
                if lo >= 0:
                    dst2 = AP(tensor=valscr2.tensor,
                              offset=valscr2[:, :].offset + lo * 2 * C + C,
                              ap=[[2 * C, 128], [128 * 2 * C, VT], [1, C]])
                    nc.sync.dma_start(dst2, src1)
                else:
                    # v == 0: rows [HW-160, HW) from (b=0 all p) + (b=1 p<32),
                    # then rows [0, 96) from (b=1 p>=32), rows [96, 480) b=2..4
                    d_a = AP(tensor=valscr2.tensor,
                             offset=valscr2[:, :].offset + (HW - W) * 2 * C + C,
                             ap=[[2 * C, 128], [1, C]])
                    s_a = AP(tensor=vsb5.tensor, offset=vsb5[:, :].offset,
                             ap=[[vsb5[:, :].ap[0][0], 128], [1, C]])
                    nc.sync.dma_start(d_a, s_a)
                    d_b = AP(tensor=valscr2.tensor,
                             offset=valscr2[:, :].offset + (HW - 32) * 2 * C + C,
                             ap=[[2 * C, 32], [1, C]])
                    s_b = AP(tensor=vsb5.tensor, offset=vsb5[:, :].offset + C,
                             ap=[[vsb5[:, :].ap[0][0], 32], [1, C]])
                    nc.sync.dma_start(d_b, s_b)
                    d_c = AP(tensor=valscr2.tensor,
                             offset=valscr2[:, :].offset + 0 * 2 * C + C,
                             ap=[[2 * C, 96], [1, C]])
                    s_c = AP(tensor=vsb5.tensor,
                             offset=vsb5[32:, :].offset + C,
                             ap=[[vsb5[:, :].ap[0][0], 96], [1, C]])
                    nc.sync.dma_start(d_c, s_c)
                    d_d = AP(tensor=valscr2.tensor,
                             offset=valscr2[:, :].offset + 96 * 2 * C + C,
                             ap=[[2 * C, 128], [128 * 2 * C, VT - 2], [1, C]])
                    s_d = AP(tensor=vsb5.tensor,
                             offset=vsb5[:, :].offset + 2 * C,
                             ap=[[vsb5[:, :].ap[0][0], 128], [C, VT - 2], [1, C]])
                    nc.sync.dma_start(d_d, s_d)
                    # pad record HW = record 0 = [V_rot(0), V_rot(W)]
                    d_p0 = AP(tensor=valscr2.tensor,
                              offset=valscr2[:, :].offset + HW * 2 * C,
                              ap=[[2 * C, 1], [1, C]])
                    s_p0 = AP(tensor=vsb5.tensor, offset=vsb5[:, :].offset,
                              ap=[[vsb5[:, :].ap[0][0], 1], [1, C]])
                    nc.sync.dma_start(d_p0, s_p0)
                    d_p1 = AP(tensor=valscr2.tensor,
                              offset=valscr2[:, :].offset + HW * 2 * C + C,
                              ap=[[2 * C, 1], [1, C]])
                    s_p1 = AP(tensor=vsb5.tensor,
                              offset=vsb5[32:, :].offset + C,
                              ap=[[vsb5[:, :].ap[0][0], 1], [1, C]])
                    nc.sync.dma_start(d_p1, s_p1)

            # ---- phase B1: softmax / bilinear weights (overlaps gathers) ----
            wx = spool.tile([128, NP_], f32)
            wy = spool.tile([128, NP_], f32)
            tt(wx[:], xp[:], xf[:], Alu.subtract)
            tt(wy[:], yp[:], yf[:], Alu.subtract)
            ux = spool.tile([128, NP_], f32)
            uy = spool.tile([128, NP_], f32)
            nc.scalar.activation(ux[:], wx[:], Act.Copy, bias=1.0, scale=-1.0)
            nc.scalar.activation(uy[:], wy[:], Act.Copy, bias=1.0, scale=-1.0)

            # softmax over points
            wmax = spool.tile([128, NT * G], f32, tag="wmax")
            wl_gp = AP(tensor=proj72.tensor, offset=proj72[:, :].offset,
                       ap=[[proj72[:, :].ap[0][0], 128], [72, NT], [1, G], [G, P]])
            nc.vector.tensor_reduce(out=wmax[:, :].rearrange("p (t g) -> p t g", g=G),
                                    in_=wl_gp, axis=Ax.X, op=Alu.max)
            smf = spool.tile([128, NT * P * G], f32, tag="smf")
            wl_pg = AP(tensor=proj72.tensor, offset=proj72[:, :].offset,
                       ap=[[proj72[:, :].ap[0][0], 128], [72, NT], [G, P], [1, G]])
            wmax_b = AP(tensor=wmax.tensor, offset=wmax[:, :].offset,
                        ap=[[wmax[:, :].ap[0][0], 128], [G, NT], [0, P], [1, G]])
            tt(smf[:, :].rearrange("p (t q g) -> p t q g", q=P, g=G),
               wl_pg, wmax_b, Alu.subtract)
            nc.scalar.activation(smf[:], smf[:], Act.Exp)
            ssum = spool.tile([128, NT * G], f32, tag="wmax")
            sm_gp = AP(tensor=smf.tensor, offset=smf[:, :].offset,
                       ap=[[smf[:, :].ap[0][0], 128], [P * G, NT], [1, G], [G, P]])
            nc.vector.tensor_reduce(out=ssum[:, :].rearrange("p (t g) -> p t g", g=G),
                                    in_=sm_gp, axis=Ax.X, op=Alu.add)
            rcps = spool.tile([128, NT * G], f32)
            nc.vector.reciprocal(rcps[:], ssum[:])
            wsm = spool.tile([128, NT * P * G], bf16)
            rcp_b = AP(tensor=rcps.tensor, offset=rcps[:, :].offset,
                       ap=[[rcps[:, :].ap[0][0], 128], [G, NT], [0, P], [1, G]])
            tt(wsm[:, :].rearrange("p (t q g) -> p t q g", q=P, g=G),
               smf[:, :].rearrange("p (t q g) -> p t q g", q=P, g=G),
               rcp_b, Alu.mult)

            # x-validity masks (with pair-clamp weight swap)
            tA = spool.tile([128, NP_], f32)
            tB = spool.tile([128, NP_], f32)
            mAx = spool.tile([128, NP_], f32)
            nc.vector.tensor_scalar(out=tA[:], in0=xf[:], scalar1=SHIFT,
                                    scalar2=None, op0=Alu.is_ge)
            nc.vector.tensor_scalar(out=tB[:], in0=xf[:], scalar1=SHIFT + W - 2,
                                    scalar2=None, op0=Alu.is_le)
            tt(mAx[:], tA[:], tB[:], Alu.mult)
            mBx = spool.tile([128, NP_], f32)
            nc.vector.tensor_scalar(out=mBx[:], in0=xf[:], scalar1=SHIFT - 1.0,
                                    scalar2=None, op0=Alu.is_equal)
            mCx = spool.tile([128, NP_], f32)
            nc.vector.tensor_scalar(out=mCx[:], in0=xf[:], scalar1=SHIFT + W - 1,
                                    scalar2=None, op0=Alu.is_equal)

            bx = spool.tile([128, NT * P * 2], f32)   # (t, pt, side)
            tt(v1[:], ux[:], mAx[:], Alu.mult)
            tt(v2[:], wx[:], mBx[:], Alu.mult)
            bx0 = AP(tensor=bx.tensor, offset=bx[:, :].offset,
                     ap=[[bx[:, :].ap[0][0], 128], [2, NP_]])
            tt(bx0, v1[:], v2[:], Alu.add)
            tt(v1[:], wx[:], mAx[:], Alu.mult)
            tt(v2[:], ux[:], mCx[:], Alu.mult)
            bx1 = AP(tensor=bx.tensor, offset=bx[:, :].offset + 1,
                     ap=[[bx[:, :].ap[0][0], 128], [2, NP_]])
            tt(bx1, v1[:], v2[:], Alu.add)

            # y masks (swap form, mirrors x): slot0 = row yg, slot1 = row yg+1
            my = spool.tile([128, NP_], f32)
            nc.vector.tensor_scalar(out=tA[:], in0=yf[:], scalar1=SHIFT,
                                    scalar2=None, op0=Alu.is_ge)
            nc.vector.tensor_scalar(out=tB[:], in0=yf[:], scalar1=SHIFT + H - 2,
                                    scalar2=None, op0=Alu.is_le)
            tt(my[:], tA[:], tB[:], Alu.mult)
            myB = spool.tile([128, NP_], f32)
            nc.vector.tensor_scalar(out=myB[:], in0=yf[:], scalar1=SHIFT - 1.0,
                                    scalar2=None, op0=Alu.is_equal)
            myC = spool.tile([128, NP_], f32)
            nc.vector.tensor_scalar(out=myC[:], in0=yf[:], scalar1=SHIFT + H - 1,
                                    scalar2=None, op0=Alu.is_equal)
            by = spool.tile([128, NT * P * 2], f32)   # (t, pt, yy)
            by0 = AP(tensor=by.tensor, offset=by[:, :].offset,
                     ap=[[by[:, :].ap[0][0], 128], [2, NP_]])
            by1 = AP(tensor=by.tensor, offset=by[:, :].offset + 1,
                     ap=[[by[:, :].ap[0][0], 128], [2, NP_]])
            tt(v1[:], uy[:], my[:], Alu.mult)
            tt(v2[:], wy[:], myB[:], Alu.mult)
            tt(by0, v1[:], v2[:], Alu.add)
            tt(v1[:], wy[:], my[:], Alu.mult)
            tt(v2[:], uy[:], myC[:], Alu.mult)
            tt(by1, v1[:], v2[:], Alu.add)

            # cw[t, pt, x, yy] = bx[t,pt,x] * by[t,pt,yy]
            cw = spool.tile([128, NT * P * 4], bf16)
            for x in range(2):
                by_v = AP(tensor=by.tensor, offset=by[:, :].offset,
                          ap=[[by[:, :].ap[0][0], 128], [2 * P, NT], [2, P], [1, 2]])
                bx_x = AP(tensor=bx.tensor, offset=bx[:, :].offset + x,
                          ap=[[bx[:, :].ap[0][0], 128], [2 * P, NT], [2, P], [0, 2]])
                cw_x = AP(tensor=cw.tensor, offset=cw[:, :].offset + 2 * x,
                          ap=[[cw[:, :].ap[0][0], 128], [4 * P, NT], [4, P], [1, 2]])
                tt(cw_x, by_v, bx_x, Alu.mult)

            # kw[t, pt, rs, g] = cw[t, pt, rs] * wsm[t, pt, g]  (bf16)
            kw = spool.tile([128, NT * P * 4 * G], bf16)
            for rs in range(4):
                cw_rs = AP(tensor=cw.tensor, offset=cw[:, :].offset + rs,
                           ap=[[cw[:, :].ap[0][0], 128], [4 * P, NT], [4, P], [0, G]])
                w_v = AP(tensor=wsm.tensor, offset=wsm[:, :].offset,
                         ap=[[wsm[:, :].ap[0][0], 128], [P * G, NT], [G, P], [1, G]])
                kw_rs = AP(tensor=kw.tensor, offset=kw[:, :].offset + rs * G,
                           ap=[[kw[:, :].ap[0][0], 128], [4 * P * G, NT], [4 * G, P], [1, G]])
                tt(kw_rs, cw_rs, w_v, Alu.mult)

            # sumcoef[t, g] = sum_pt wsm * (bx0+bx1)*(by0+by1)   (for value_b)
            bx0r = AP(tensor=bx.tensor, offset=bx[:, :].offset,
                      ap=[[bx[:, :].ap[0][0], 128], [2, NP_]])
            bx1r = AP(tensor=bx.tensor, offset=bx[:, :].offset + 1,
                      ap=[[bx[:, :].ap[0][0], 128], [2, NP_]])
            by0r = AP(tensor=by.tensor, offset=by[:, :].offset,
                      ap=[[by[:, :].ap[0][0], 128], [2, NP_]])
            by1r = AP(tensor=by.tensor, offset=by[:, :].offset + 1,
                      ap=[[by[:, :].ap[0][0], 128], [2, NP_]])
            tt(v1[:], bx0r, bx1r, Alu.add)
            tt(v2[:], by0r, by1r, Alu.add)
            bws = spool.tile([128, NP_], bf16)
            tt(bws[:], v1[:], v2[:], Alu.mult)
            wp = spool.tile([128, NT * P * G], bf16, tag="smf")  # reuse smf slot
            bws_b = AP(tensor=bws.tensor, offset=bws[:, :].offset,
                       ap=[[bws[:, :].ap[0][0], 128], [P, NT], [1, P], [0, G]])
            tt(wp[:, :].rearrange("p (t q g) -> p t q g", q=P, g=G),
               wsm[:, :].rearrange("p (t q g) -> p t q g", q=P, g=G),
               bws_b, Alu.mult)
            sumcoef = spool.tile([128, NT * G], f32)
            wp_gp = AP(tensor=wp.tensor, offset=wp[:, :].offset,
                       ap=[[wp[:, :].ap[0][0], 128], [P * G, NT], [1, G], [G, P]])
            nc.vector.tensor_reduce(out=sumcoef[:, :].rearrange("p (t g) -> p t g", g=G),
                                    in_=wp_gp, axis=Ax.X, op=Alu.add)

            # ---- phase C: gather chunks + combine ----
            val_src = AP(tensor=valscr2.tensor, offset=valscr2[:, :].offset,
                         ap=[[2 * C, HW], [1, 4 * C]])

            for ch, (t0, ntile) in enumerate(CHUNKS):
                nidx = ntile * P * 128
                gt = gpool.tile([128, TCH * P, 4 * C], bf16, tag="g", bufs=3,
                                name=f"g{ch}")
                nc.gpsimd.dma_gather(
                    gt[:, : ntile * P, :], val_src,
                    idx16[:, COLOFF[ch] : COLOFF[ch] + nidx // 16],
                    num_idxs=nidx, num_idxs_reg=nidx,
                    elem_size=4 * C, elem_step=2 * C, single_packet=False,
                )

                for tt_ in range(ntile):
                    t = t0 + tt_
                    # tp[q, (j36, c)] = gt * kw  (j36 = (pt, x, yy), c = (g, gc))
                    tp = tpool.tile([128, P * 4 * C], bf16, tag="tp", bufs=1,
                                    name=f"tp{t}")
                    g_v = AP(tensor=gt.tensor,
                             offset=gt[:, :, :].offset + tt_ * P * 4 * C,
                             ap=[[gt[:, :, :].ap[0][0], 128],
                                 [C, P * 4], [GC, G], [1, GC]])
                    kw_v = AP(tensor=kw.tensor, offset=kw[:, :].offset + t * P * 4 * G,
                              ap=[[kw[:, :].ap[0][0], 128],
                                  [G, P * 4], [1, G], [0, GC]])
                    tp_v = AP(tensor=tp.tensor, offset=tp[:, :].offset,
                              ap=[[tp[:, :].ap[0][0], 128],
                                  [C, P * 4], [GC, G], [1, GC]])
                    nc.vector.tensor_tensor(out=tp_v, in0=g_v, in1=kw_v, op=Alu.mult)

                    # pre-add yy pairs (2x mode): tp2[q, ((pt,x), c)]
                    tp2 = tpool.tile([128, P * 2 * C], bf16, tag="tp2", name=f"tp2{t}")
                    in0 = AP(tensor=tp.tensor, offset=tp[:, :].offset,
                             ap=[[tp[:, :].ap[0][0], 128], [2 * C, 2 * P], [1, C]])
                    in1 = AP(tensor=tp.tensor, offset=tp[:, :].offset + C,
                             ap=[[tp[:, :].ap[0][0], 128], [2 * C, 2 * P], [1, C]])
                    out2 = AP(tensor=tp2.tensor, offset=tp2[:, :].offset,
                              ap=[[tp2[:, :].ap[0][0], 128], [C, 2 * P], [1, C]])
                    nc.vector.tensor_tensor(out=out2, in0=in0, in1=in1, op=Alu.add)

                    # ebias[q, c] = value_b[c] * sumcoef[q, g(c)]  (bf16)
                    ebias = apool.tile([128, C], bf16, tag="eb", bufs=1, name=f"eb{t}")
                    sc_v = AP(tensor=sumcoef.tensor,
                              offset=sumcoef[:, :].offset + t * G,
                              ap=[[sumcoef[:, :].ap[0][0], 128], [1, G], [0, GC]])
                    bv_v = bvr_s[:, :].rearrange("p (g c) -> p g c", g=G)
                    nc.vector.tensor_tensor(
                        out=ebias[:, :].rearrange("p (g c) -> p g c", g=G),
                        in0=sc_v, in1=bv_v, op=Alu.mult)

                    # 19 transpose-accumulate matmuls: aggT[c, q] in PSUM
                    aggT = pspool.tile([128, 128], f32, tag="mm128", bufs=2,
                                       name=f"aggT{t}")
                    for k in range(2 * P):
                        nc.tensor.matmul(aggT[:], tp2[:, k * C : (k + 1) * C],
                                         ident_s[:], start=(k == 0), stop=False)
                    nc.tensor.matmul(aggT[:], ebias[:], ident_s[:],
                                     start=False, stop=True)

                    # out-projection straight off the accumulated PSUM
                    aT = apool.tile([128, 128], bf16, tag="aT", bufs=1, name=f"aT{t}")
                    nc.scalar.copy(aT[:], aggT[:])
                    fops = pspool.tile([128, C], f32, tag="fo", bufs=2,
                                       name=f"fo{t}")
                    nc.tensor.matmul(fops[:], owT_s[:], aT[:], start=True, stop=True)
                    fo_sb = apool.tile([128, C], f32, tag="fosb", name=f"fosb{t}")
                    nc.scalar.activation(fo_sb[:], fops[:], Act.Identity,
                                         bias=outb_s[:, 0:1], scale=1.0)
                    nc.sync.dma_start(out_d[:, t * 128 : (t + 1) * 128], fo_sb[:])

    nc.finalize()
    return nc


def _host_prep(inputs):
    """Prepare per-core input maps from full inputs."""
    feats = np.asarray(inputs["feats"], np.float32)          # [B, C, H, W]
    anchor = np.asarray(inputs["anchor_points"], np.float32)  # [B, HW, 2]
    value_w = np.asarray(inputs["value_w"], np.float32)
    value_b = np.asarray(inputs["value_b"], np.float32)
    weights_w = np.asarray(inputs["weights_w"], np.float32)
    weights_b = np.asarray(inputs["weights_b"], np.float32)
    offset_w = np.asarray(inputs["offset_w"], np.float32)
    offset_b = np.asarray(inputs["offset_b"], np.float32)
    out_w = np.asarray(inputs["out_w"], np.float32)
    out_b = np.asarray(inputs["out_b"], np.float32)

    shared = {
        "vwT16": np.ascontiguousarray(value_w.T).astype(ml_dtypes.bfloat16),
        "w72T": np.ascontiguousarray(weights_w.T).astype(ml_dtypes.bfloat16),
        "w18T": np.ascontiguousarray(offset_w.T),
        "owT16": np.ascontiguousarray(out_w.T).astype(ml_dtypes.bfloat16),
        "b72r": np.broadcast_to(weights_b, (128, 72)).copy(),
        "b18r": np.broadcast_to(offset_b, (128, 18)).copy(),
        "bvr": np.broadcast_to(value_b, (128, C)).copy(),
        "outb": out_b.reshape(128, 1).copy(),
        "ident16": np.eye(128, dtype=ml_dtypes.bfloat16),
    }
    oneh = np.zeros((128, 8, 128), np.float32)
    for qh in range(8):
        for m in range(128):
            oneh[16 * qh + (m % 16), qh, m] = 1.0
    shared["oneh"] = oneh.reshape(128, 8 * 128)

    in_maps = []
    feats16_b = [feats[b].reshape(C, HW).astype(ml_dtypes.bfloat16) for b in range(B)]
    for core in range(NCORES):
        b_i, sl = core // 4, core % 4
        off = sl * QS
        f16 = np.roll(feats16_b[b_i], -off, axis=1)
        f32r = np.roll(feats[b_i].reshape(C, HW), -off, axis=1)[:, :QS]
        an = anchor[b_i, off : off + QS].reshape(NT, 128, 2).transpose(1, 0, 2).reshape(128, NT * 2)
        m = dict(shared)
        m["feats16"] = np.ascontiguousarray(f16)
        m["feats32"] = np.ascontiguousarray(f32r)
        m["anch"] = np.ascontiguousarray(an)
        m["rotoff"] = np.full((128, 1), float(off), np.float32)
        in_maps.append(m)
    return in_maps


def kernel(**inputs) -> np.ndarray:
    from concourse.bass_utils import run_bass_kernel_spmd

    if "nc" not in _CACHE:
        _CACHE["nc"] = _build_nc()
    nc = _CACHE["nc"]
    in_maps = _host_prep(inputs)
    res = run_bass_kernel_spmd(nc, in_maps, core_ids=list(range(NCORES)))
    out = np.zeros((B, C, HW), np.float32)
    for core in range(NCORES):
        b_i, sl = core // 4, core % 4
        out[b_i, :, sl * QS : (sl + 1) * QS] = res.results[core]["out"]
    return out.reshape(B, C, H, W)


# revision 17
# speedup vs baseline: 1.9648x; 1.0572x over previous
"""Deformable 2D feature aggregator — Trainium2 Bass kernel, 8-core SPMD. v3.

Problem: B=2, C=128, H=96, W=160, P=9 points, G=8 groups.
  value = conv1x1(feats); w = softmax over P of conv1x1(feats); offs = conv1x1(feats)
  pts = anchors + offs; out_proj(conv-weighted bilinear gather of value at pts).

Sharding: 8 cores = 2 batches x 4 query-slices; each core builds the full
(rotated) value map for its batch, then gathers per-query corner windows.

v3 vs v2:
  - coords -> idxf -> idx16 chain emitted FIRST so the gather stream (gated on
    idx16 + valscr2) starts ~50us earlier; softmax/kw overlap under gathers.
  - last two gather chunks are single-tile so the final combine tail is half.
  - gather pool triple-buffered to absorb combine latency between chunks.
"""
import sys

sys.path.insert(0, "/opt/trn_rl_repo")

import numpy as np
import ml_dtypes

import concourse.bass as bass
import concourse.bacc as bacc
import concourse.mybir as mybir
import concourse.tile as tile
from concourse import library_config
from concourse.ap import AP

# problem constants (hardcoded per harness contract)
B, C, H, W = 2, 128, 96, 160
HW = H * W                     # 15360
P, G, GC = 9, 8, 16
NCORES = 8
QS = B * HW // NCORES          # 3840 queries per core
NT = QS // 128                 # 30 query tiles
TCH = 2                        # query tiles per gather chunk
NCH = NT // TCH                # 15 gather chunks
NP_ = NT * P                   # 270 points per partition-row
NIDX_CH = TCH * 128 * P        # 2304 gather indices per chunk
SHIFT = 1024.0                 # floor-bias (exact in f32 for our range)
NPXT = HW // 128               # 120 pixel tiles
VT = 12                        # pixel tiles per valscr2 write chunk
NCHV = NPXT // VT              # 24 write chunks

f32 = mybir.dt.float32
bf16 = mybir.dt.bfloat16
i16 = mybir.dt.int16
Alu = mybir.AluOpType
Act = mybir.ActivationFunctionType
Ax = mybir.AxisListType

_CACHE: dict = {}


def _build_nc():
    nc = bacc.Bacc()

    feats16 = nc.dram_tensor("feats16", [C, HW], bf16, kind="ExternalInput")
    feats32 = nc.dram_tensor("feats32", [C, QS], f32, kind="ExternalInput")
    anch = nc.dram_tensor("anch", [128, NT * 2], f32, kind="ExternalInput")
    vwT16 = nc.dram_tensor("vwT16", [C, C], bf16, kind="ExternalInput")
    w72T = nc.dram_tensor("w72T", [C, 72], bf16, kind="ExternalInput")
    w18T = nc.dram_tensor("w18T", [C, 18], f32, kind="ExternalInput")
    owT16 = nc.dram_tensor("owT16", [C, C], bf16, kind="ExternalInput")
    b72r = nc.dram_tensor("b72r", [128, 72], f32, kind="ExternalInput")
    b18r = nc.dram_tensor("b18r", [128, 18], f32, kind="ExternalInput")
    bvr = nc.dram_tensor("bvr", [128, C], f32, kind="ExternalInput")
    outb = nc.dram_tensor("outb", [128, 1], f32, kind="ExternalInput")
    oneh = nc.dram_tensor("oneh", [128, 8 * 128], f32, kind="ExternalInput")
    ident16 = nc.dram_tensor("ident16", [128, 128], bf16, kind="ExternalInput")
    rotoff = nc.dram_tensor("rotoff", [128, 1], f32, kind="ExternalInput")
    out_d = nc.dram_tensor("out", [C, QS], f32, kind="ExternalOutput")

    with tile.TileContext(nc) as tc, nc.allow_low_precision("bf16 combine by design"):
        with (
            tc.tile_pool(name="const", bufs=1) as cpool,
            tc.tile_pool(name="stage", bufs=1) as spool,
            tc.tile_pool(name="vsb", bufs=3) as vpool,
            tc.tile_pool(name="g", bufs=2) as gpool,
            tc.tile_pool(name="tprime", bufs=2) as tpool,
            tc.tile_pool(name="aggp", bufs=2) as apool,
            tc.tile_pool(name="ps", bufs=1, space="PSUM") as pspool,
            tc.tile_pool(name="dram", bufs=1, space="DRAM") as dpool,
        ):
            # ---- persistent loads (query-path tensors first: they gate idx16) ----
            f32s = spool.tile([128, QS], f32)
            nc.sync.dma_start(f32s[:], feats32[:])
            w72T_s = cpool.tile([C, 72], bf16)
            nc.sync.dma_start(w72T_s[:], w72T[:])
            w18T_s = cpool.tile([C, 18], f32)
            nc.sync.dma_start(w18T_s[:], w18T[:])
            anch_s = cpool.tile([128, NT * 2], f32)
            nc.sync.dma_start(anch_s[:], anch[:])
            rot_s = cpool.tile([128, 1], f32)
            nc.sync.dma_start(rot_s[:], rotoff[:])
            oneh_s = cpool.tile([128, 8 * 128], f32)
            nc.sync.dma_start(oneh_s[:], oneh[:])
            b72_s = cpool.tile([128, 72], f32)
            nc.sync.dma_start(b72_s[:], b72r[:])
            b18_s = cpool.tile([128, 18], f32)
            nc.sync.dma_start(b18_s[:], b18r[:])
            f16s = spool.tile([128, HW], bf16)
            nc.sync.dma_start(f16s[:], feats16[:])
            vwT_s = cpool.tile([C, C], bf16)
            nc.sync.dma_start(vwT_s[:], vwT16[:])
            owT_s = cpool.tile([C, C], bf16)
            nc.sync.dma_start(owT_s[:], owT16[:])
            bvr_s = cpool.tile([128, C], f32)
            nc.sync.dma_start(bvr_s[:], bvr[:])
            outb_s = cpool.tile([128, 1], f32)
            nc.sync.dma_start(outb_s[:], outb[:])
            ident_s = cpool.tile([128, 128], bf16)
            nc.sync.dma_start(ident_s[:], ident16[:])

            # dummy gather: preloads the GPSIMD gather-library IRAM during
            # the head so the first real gather doesn't pay the ~12us load.
            dumscr = dpool.tile([17, 2 * C], bf16)
            dumsrc = AP(tensor=dumscr.tensor, offset=dumscr[:, :].offset,
                        ap=[[2 * C, 16], [1, 4 * C]])
            dumidx = spool.tile([128, 1], i16)
            nc.vector.memset(dumidx[:], 0)
            dumout = spool.tile([128, 1, 4 * C], bf16)
            nc.gpsimd.dma_gather(
                dumout[:, :, :], dumsrc, dumidx[:, 0:1],
                num_idxs=16, num_idxs_reg=16,
                elem_size=4 * C, elem_step=2 * C, single_packet=False,
            )

            # pair-row scratch: record r = [V_rot(r), V_rot((r+W) mod HW)].
            # +1 pad record (= record 0) backs the r0+1 read at r0 = HW-1,
            # which is reachable after rotation.
            valscr2 = dpool.tile([HW + 1, 2 * C], bf16)

            proj72 = spool.tile([128, NT * 72], bf16)
            proj18 = spool.tile([128, NT * 18], f32)

            def tt(out, in0, in1, op):
                nc.vector.tensor_tensor(out=out, in0=in0, in1=in1, op=op)

            # ---- phase A0: query projections (offsets f32; wlog bf16) ----
            for t in range(NT):
                pp18 = pspool.tile([128, 18], f32, tag="pp", bufs=2,
                                   name=f"pp18_{t}")
                nc.tensor.matmul(pp18[:], f32s[:, t * 128 : (t + 1) * 128],
                                 w18T_s[:], start=True, stop=True)
                nc.vector.tensor_tensor(
                    out=proj18[:, t * 18 : (t + 1) * 18],
                    in0=pp18[:], in1=b18_s[:], op=Alu.add)
            for t in range(NT):
                pp72 = pspool.tile([128, 72], f32, tag="pp", bufs=2,
                                   name=f"pp72_{t}")
                nc.tensor.matmul(pp72[:], f16s[:, t * 128 : (t + 1) * 128],
                                 w72T_s[:], start=True, stop=True)
                nc.vector.tensor_tensor(
                    out=proj72[:, t * 72 : (t + 1) * 72],
                    in0=pp72[:], in1=b72_s[:], op=Alu.add)

            # ---- phase B0: coords -> gather record indices (critical path) ----
            px = spool.tile([128, NP_], f32, tag="px")
            py = spool.tile([128, NP_], f32, tag="py")
            offs_x = AP(tensor=proj18.tensor, offset=proj18[:, :].offset,
                        ap=[[proj18[:, :].ap[0][0], 128], [18, NT], [2, P]])
            offs_y = AP(tensor=proj18.tensor, offset=proj18[:, :].offset + 1,
                        ap=[[proj18[:, :].ap[0][0], 128], [18, NT], [2, P]])
            anx = AP(tensor=anch_s.tensor, offset=anch_s[:, :].offset,
                     ap=[[anch_s[:, :].ap[0][0], 128], [2, NT], [0, P]])
            any_ = AP(tensor=anch_s.tensor, offset=anch_s[:, :].offset + 1,
                      ap=[[anch_s[:, :].ap[0][0], 128], [2, NT], [0, P]])
            tt(px[:, :].rearrange("p (t q) -> p t q", q=P), offs_x, anx, Alu.add)
            tt(py[:, :].rearrange("p (t q) -> p t q", q=P), offs_y, any_, Alu.add)

            xp = spool.tile([128, NP_], f32)
            yp = spool.tile([128, NP_], f32)
            nc.scalar.activation(xp[:], px[:], Act.Copy, bias=SHIFT - 0.5, scale=float(W))
            nc.scalar.activation(yp[:], py[:], Act.Copy, bias=SHIFT - 0.5, scale=float(H))
            # floor via round(x-0.5): (x + (2^23-0.5)) - 2^23. At integer x the
            # half-even tie may floor one low with frac 1.0 — an equivalent
            # bilinear weighting, so interpolation is unchanged.
            MAGIC = float(1 << 23)
            xf = spool.tile([128, NP_], f32, tag="px")   # reuse px slot
            yf = spool.tile([128, NP_], f32, tag="py")   # reuse py slot
            nc.vector.tensor_scalar(out=xf[:], in0=xp[:], scalar1=MAGIC - 0.5,
                                    scalar2=MAGIC, op0=Alu.add, op1=Alu.subtract)
            nc.vector.tensor_scalar(out=yf[:], in0=yp[:], scalar1=MAGIC - 0.5,
                                    scalar2=MAGIC, op0=Alu.add, op1=Alu.subtract)

            xg = spool.tile([128, NP_], f32)
            nc.vector.tensor_scalar(out=xg[:], in0=xf[:], scalar1=SHIFT, scalar2=0.0,
                                    op0=Alu.subtract, op1=Alu.max)
            nc.vector.tensor_scalar(out=xg[:], in0=xg[:], scalar1=float(W - 2),
                                    scalar2=None, op0=Alu.min)
            yg = spool.tile([128, NP_], f32)
            nc.vector.tensor_scalar(out=yg[:], in0=yf[:], scalar1=SHIFT, scalar2=0.0,
                                    op0=Alu.subtract, op1=Alu.max)
            nc.vector.tensor_scalar(out=yg[:], in0=yg[:], scalar1=float(H - 2),
                                    scalar2=None, op0=Alu.min)

            # idx = (yg*W + xg - rotoff) mod HW
            v1 = spool.tile([128, NP_], f32)
            v2 = spool.tile([128, NP_], f32)
            idxf = spool.tile([128, NP_], f32)
            nc.scalar.activation(v1[:], yg[:], Act.Copy, bias=0.0, scale=float(W))
            tt(idxf[:], v1[:], xg[:], Alu.add)
            nc.vector.tensor_scalar(out=idxf[:], in0=idxf[:], scalar1=rot_s[:, 0:1],
                                    scalar2=None, op0=Alu.subtract)
            nc.vector.tensor_scalar(out=v2[:], in0=idxf[:], scalar1=0.0,
                                    scalar2=float(HW), op0=Alu.is_lt, op1=Alu.mult)
            tt(idxf[:], idxf[:], v2[:], Alu.add)

            # ---- phase A1: bf16 value map over the whole rotated image ----
            def emit_value_chunk(v):
                vsb5 = vpool.tile([128, VT * C], bf16, tag="vsb5", bufs=2,
                                  name=f"vsb5_{v}")
                for k4 in range(VT // 4):
                    t = v * VT + 4 * k4
                    vps = pspool.tile([128, 4 * C], f32, tag="vps", bufs=3,
                                      name=f"vps{t}")
                    for j in range(4):
                        nc.tensor.matmul(
                            vps[:, j * C : (j + 1) * C],
                            f16s[:, (t + j) * 128 : (t + j + 1) * 128],
                            vwT_s[:], start=True, stop=True)
                    nc.scalar.copy(vsb5[:, 4 * k4 * C : (4 * k4 + 4) * C], vps[:])
                base = v * VT * 128  # first pixel (row) of this chunk
                # first half: rows [base, base+640), cols 0:C
                dst1 = AP(tensor=valscr2.tensor,
                          offset=valscr2[:, :].offset + base * 2 * C,
                          ap=[[2 * C, 128], [128 * 2 * C, VT], [1, C]])
                src1 = AP(tensor=vsb5.tensor, offset=vsb5[:, :].offset,
                          ap=[[vsb5[:, :].ap[0][0], 128], [C, VT], [1, C]])
                nc.sync.dma_start(dst1, src1)
                # second half: rows [(base - W) mod HW ...), cols C:2C
                lo = base - W
                if lo >= 0:
                    dst2 = AP(tensor=valscr2.tensor,
                              offset=valscr2[:, :].offset + lo * 2 * C + C,
                              ap=[[2 * C, 128], [128 * 2 * C, VT], [1, C]])
                    nc.sync.dma_start(dst2, src1)
                else:
                    # v == 0: rows [HW-160, HW) from (b=0 all p) + (b=1 p<32),
                    # then rows [0, 96) from (b=1 p>=32), rows [96, 480) b=2..4
                    d_a = AP(tensor=valscr2.tensor,
                             offset=valscr2[:, :].offset + (HW - W) * 2 * C + C,
                             ap=[[2 * C, 128], [1, C]])
                    s_a = AP(tensor=vsb5.tensor, offset=vsb5[:, :].offset,
                             ap=[[vsb5[:, :].ap[0][0], 128], [1, C]])
                    nc.sync.dma_start(d_a, s_a)
                    d_b = AP(tensor=valscr2.tensor,
                             offset=valscr2[:, :].offset + (HW - 32) * 2 * C + C,
                             ap=[[2 * C, 32], [1, C]])
                    s_b = AP(tensor=vsb5.tensor, offset=vsb5[:, :].offset + C,
                             ap=[[vsb5[:, :].ap[0][0], 32], [1, C]])
                    nc.sync.dma_start(d_b, s_b)
                    d_c = AP(tensor=valscr2.tensor,
                             offset=valscr2[:, :].offset + 0 * 2 * C + C,
                             ap=[[2 * C, 96], [1, C]])
                    s_c = AP(tensor=vsb5.tensor,
                             offset=vsb5[32:, :].offset + C,
                             ap=[[vsb5[:, :].ap[0][0], 96], [1, C]])
                    nc.sync.dma_start(d_c, s_c)
                    d_d = AP(tensor=valscr2.tensor,
                             offset=valscr2[:, :].offset + 96 * 2 * C + C,
                             ap=[[2 * C, 128], [128 * 2 * C, VT - 2], [1, C]])
                    s_d = AP(tensor=vsb5.tensor,
                             offset=vsb5[:, :].offset + 2 * C,
                             ap=[[vsb5[:, :].ap[0][0], 128], [C, VT - 2], [1, C]])
                    nc.sync.dma_start(d_d, s_d)
                    # pad record HW = record 0 = [V_rot(0), V_rot(W)]
                    d_p0 = AP(tensor=valscr2.tensor,
                              offset=valscr2[:, :].offset + HW * 2 * C,
                              ap=[[2 * C, 1], [1, C]])
                    s_p0 = AP(tensor=vsb5.tensor, offset=vsb5[:, :].offset,
                              ap=[[vsb5[:, :].ap[0][0], 1], [1, C]])
                    nc.sync.dma_start(d_p0, s_p0)
                    d_p1 = AP(tensor=valscr2.tensor,
                              offset=valscr2[:, :].offset + HW * 2 * C + C,
                              ap=[[2 * C, 1], [1, C]])
                    s_p1 = AP(tensor=vsb5.tensor,
                              offset=vsb5[32:, :].offset + C,
                              ap=[[vsb5[:, :].ap[0][0], 1], [1, C]])
                    nc.sync.dma_start(d_p1, s_p1)


            for v in range(3):
                emit_value_chunk(v)

            # chunk schedule: 14 double-tile chunks + 2 single-tile chunks
            # (small tail chunks shorten the final gather-drain + combine).
            CHUNKS = [(2 * i, 2) for i in range(14)] + [(28, 1), (29, 1)]
            COLOFF = []
            acc = 0
            for (_, ntile) in CHUNKS:
                COLOFF.append(acc)
                acc += ntile * P * 8

            # idx16: position i in chunk = jj*128 + q (jj = tt*P + pt),
            # stored at idx16[q%16 (+16a), coloff + 8*jj + q//16]
            idx16 = spool.tile([128, NT * P * 8], i16)
            for qh in range(8):
                i16ps = pspool.tile([128, NP_], f32, tag="pp", bufs=2,
                                    name=f"i16ps{qh}")
                nc.tensor.matmul(i16ps[:], oneh_s[:, qh * 128 : (qh + 1) * 128],
                                 idxf[:], start=True, stop=True)
                dst = AP(tensor=idx16.tensor, offset=idx16[:, :].offset + qh,
                         ap=[[idx16[:, :].ap[0][0], 128],
                             [2 * P * 8, 14], [P * 8, 2], [8, P]])
                src = AP(tensor=i16ps.tensor, offset=i16ps[:, :].offset,
                         ap=[[i16ps[:, :].ap[0][0], 128],
                             [2 * P, 14], [P, 2], [1, P]])
                nc.vector.tensor_copy(dst, src)
                # tail tiles 28, 29 (single-tile chunks, contiguous blocks)
                dst_t = AP(tensor=idx16.tensor,
                           offset=idx16[:, :].offset + qh + 28 * P * 8,
                           ap=[[idx16[:, :].ap[0][0], 128], [P * 8, 2], [8, P]])
                src_t = AP(tensor=i16ps.tensor,
                           offset=i16ps[:, :].offset + 28 * P,
                           ap=[[i16ps[:, :].ap[0][0], 128], [P, 2], [1, P]])
                nc.vector.tensor_copy(dst_t, src_t)

            for v in range(3, NCHV):
                emit_value_chunk(v)

            # ---- phase B1: softmax / bilinear weights (overlaps gathers) ----
            wx = spool.tile([128, NP_], f32)
            wy = spool.tile([128, NP_], f32)
            tt(wx[:], xp[:], xf[:], Alu.subtract)
            tt(wy[:], yp[:], yf[:], Alu.subtract)
            ux = spool.tile([128, NP_], f32)
            uy = spool.tile([128, NP_], f32)
            nc.scalar.activation(ux[:], wx[:], Act.Copy, bias=1.0, scale=-1.0)
            nc.scalar.activation(uy[:], wy[:], Act.Copy, bias=1.0, scale=-1.0)

            # softmax over points
            wmax = spool.tile([128, NT * G], f32, tag="wmax")
            wl_gp = AP(tensor=proj72.tensor, offset=proj72[:, :].offset,
                       ap=[[proj72[:, :].ap[0][0], 128], [72, NT], [1, G], [G, P]])
            nc.vector.tensor_reduce(out=wmax[:, :].rearrange("p (t g) -> p t g", g=G),
                                    in_=wl_gp, axis=Ax.X, op=Alu.max)
            smf = spool.tile([128, NT * P * G], f32, tag="smf")
            wl_pg = AP(tensor=proj72.tensor, offset=proj72[:, :].offset,
                       ap=[[proj72[:, :].ap[0][0], 128], [72, NT], [G, P], [1, G]])
            wmax_b = AP(tensor=wmax.tensor, offset=wmax[:, :].offset,
                        ap=[[wmax[:, :].ap[0][0], 128], [G, NT], [0, P], [1, G]])
            tt(smf[:, :].rearrange("p (t q g) -> p t q g", q=P, g=G),
               wl_pg, wmax_b, Alu.subtract)
            nc.scalar.activation(smf[:], smf[:], Act.Exp)
            ssum = spool.tile([128, NT * G], f32, tag="wmax")
            sm_gp = AP(tensor=smf.tensor, offset=smf[:, :].offset,
                       ap=[[smf[:, :].ap[0][0], 128], [P * G, NT], [1, G], [G, P]])
            nc.vector.tensor_reduce(out=ssum[:, :].rearrange("p (t g) -> p t g", g=G),
                                    in_=sm_gp, axis=Ax.X, op=Alu.add)
            rcps = spool.tile([128, NT * G], f32)
            nc.vector.reciprocal(rcps[:], ssum[:])
            wsm = spool.tile([128, NT * P * G], bf16)
            rcp_b = AP(tensor=rcps.tensor, offset=rcps[:, :].offset,
                       ap=[[rcps[:, :].ap[0][0], 128], [G, NT], [0, P], [1, G]])
            tt(wsm[:, :].rearrange("p (t q g) -> p t q g", q=P, g=G),
               smf[:, :].rearrange("p (t q g) -> p t q g", q=P, g=G),
               rcp_b, Alu.mult)

            # x-validity masks (with pair-clamp weight swap)
            tA = spool.tile([128, NP_], f32)
            tB = spool.tile([128, NP_], f32)
            mAx = spool.tile([128, NP_], f32)
            nc.vector.tensor_scalar(out=tA[:], in0=xf[:], scalar1=SHIFT,
                                    scalar2=None, op0=Alu.is_ge)
            nc.vector.tensor_scalar(out=tB[:], in0=xf[:], scalar1=SHIFT + W - 2,
                                    scalar2=None, op0=Alu.is_le)
            tt(mAx[:], tA[:], tB[:], Alu.mult)
            mBx = spool.tile([128, NP_], f32)
            nc.vector.tensor_scalar(out=mBx[:], in0=xf[:], scalar1=SHIFT - 1.0,
                                    scalar2=None, op0=Alu.is_equal)
            mCx = spool.tile([128, NP_], f32)
            nc.vector.tensor_scalar(out=mCx[:], in0=xf[:], scalar1=SHIFT + W - 1,
                                    scalar2=None, op0=Alu.is_equal)

            bx = spool.tile([128, NT * P * 2], f32)   # (t, pt, side)
            tt(v1[:], ux[:], mAx[:], Alu.mult)
            tt(v2[:], wx[:], mBx[:], Alu.mult)
            bx0 = AP(tensor=bx.tensor, offset=bx[:, :].offset,
                     ap=[[bx[:, :].ap[0][0], 128], [2, NP_]])
            tt(bx0, v1[:], v2[:], Alu.add)
            tt(v1[:], wx[:], mAx[:], Alu.mult)
            tt(v2[:], ux[:], mCx[:], Alu.mult)
            bx1 = AP(tensor=bx.tensor, offset=bx[:, :].offset + 1,
                     ap=[[bx[:, :].ap[0][0], 128], [2, NP_]])
            tt(bx1, v1[:], v2[:], Alu.add)

            # y masks (swap form, mirrors x): slot0 = row yg, slot1 = row yg+1
            my = spool.tile([128, NP_], f32)
            nc.vector.tensor_scalar(out=tA[:], in0=yf[:], scalar1=SHIFT,
                                    scalar2=None, op0=Alu.is_ge)
            nc.vector.tensor_scalar(out=tB[:], in0=yf[:], scalar1=SHIFT + H - 2,
                                    scalar2=None, op0=Alu.is_le)
            tt(my[:], tA[:], tB[:], Alu.mult)
            myB = spool.tile([128, NP_], f32)
            nc.vector.tensor_scalar(out=myB[:], in0=yf[:], scalar1=SHIFT - 1.0,
                                    scalar2=None, op0=Alu.is_equal)
            myC = spool.tile([128, NP_], f32)
            nc.vector.tensor_scalar(out=myC[:], in0=yf[:], scalar1=SHIFT + H - 1,
                                    scalar2=None, op0=Alu.is_equal)
            by = spool.tile([128, NT * P * 2], f32)   # (t, pt, yy)
            by0 = AP(tensor=by.tensor, offset=by[:, :].offset,
                     ap=[[by[:, :].ap[0][0], 128], [2, NP_]])
            by1 = AP(tensor=by.tensor, offset=by[:, :].offset + 1,
                     ap=[[by[:, :].ap[0][0], 128], [2, NP_]])
            tt(v1[:], uy[:], my[:], Alu.mult)
            tt(v2[:], wy[:], myB[:], Alu.mult)
            tt(by0, v1[:], v2[:], Alu.add)
            tt(v1[:], wy[:], my[:], Alu.mult)
            tt(v2[:], uy[:], myC[:], Alu.mult)
            tt(by1, v1[:], v2[:], Alu.add)

            # cw[t, pt, x, yy] = bx[t,pt,x] * by[t,pt,yy]
            cw = spool.tile([128, NT * P * 4], bf16)
            for x in range(2):
                by_v = AP(tensor=by.tensor, offset=by[:, :].offset,
                          ap=[[by[:, :].ap[0][0], 128], [2 * P, NT], [2, P], [1, 2]])
                bx_x = AP(tensor=bx.tensor, offset=bx[:, :].offset + x,
                          ap=[[bx[:, :].ap[0][0], 128], [2 * P, NT], [2, P], [0, 2]])
                cw_x = AP(tensor=cw.tensor, offset=cw[:, :].offset + 2 * x,
                          ap=[[cw[:, :].ap[0][0], 128], [4 * P, NT], [4, P], [1, 2]])
                tt(cw_x, by_v, bx_x, Alu.mult)

            # kw[t, pt, rs, g] = cw[t, pt, rs] * wsm[t, pt, g]  (bf16)
            kw = spool.tile([128, NT * P * 4 * G], bf16)
            for rs in range(4):
                cw_rs = AP(tensor=cw.tensor, offset=cw[:, :].offset + rs,
                           ap=[[cw[:, :].ap[0][0], 128], [4 * P, NT], [4, P], [0, G]])
                w_v = AP(tensor=wsm.tensor, offset=wsm[:, :].offset,
                         ap=[[wsm[:, :].ap[0][0], 128], [P * G, NT], [G, P], [1, G]])
                kw_rs = AP(tensor=kw.tensor, offset=kw[:, :].offset + rs * G,
                           ap=[[kw[:, :].ap[0][0], 128], [4 * P * G, NT], [4 * G, P], [1, G]])
                tt(kw_rs, cw_rs, w_v, Alu.mult)

            # sumcoef[t, g] = sum_pt wsm * (bx0+bx1)*(by0+by1)   (for value_b)
            bx0r = AP(tensor=bx.tensor, offset=bx[:, :].offset,
                      ap=[[bx[:, :].ap[0][0], 128], [2, NP_]])
            bx1r = AP(tensor=bx.tensor, offset=bx[:, :].offset + 1,
                      ap=[[bx[:, :].ap[0][0], 128], [2, NP_]])
            by0r = AP(tensor=by.tensor, offset=by[:, :].offset,
                      ap=[[by[:, :].ap[0][0], 128], [2, NP_]])
            by1r = AP(tensor=by.tensor, offset=by[:, :].offset + 1,
                      ap=[[by[:, :].ap[0][0], 128], [2, NP_]])
            tt(v1[:], bx0r, bx1r, Alu.add)
            tt(v2[:], by0r, by1r, Alu.add)
            bws = spool.tile([128, NP_], bf16)
            tt(bws[:], v1[:], v2[:], Alu.mult)
            wp = spool.tile([128, NT * P * G], bf16, tag="smf")  # reuse smf slot
            bws_b = AP(tensor=bws.tensor, offset=bws[:, :].offset,
                       ap=[[bws[:, :].ap[0][0], 128], [P, NT], [1, P], [0, G]])
            tt(wp[:, :].rearrange("p (t q g) -> p t q g", q=P, g=G),
               wsm[:, :].rearrange("p (t q g) -> p t q g", q=P, g=G),
               bws_b, Alu.mult)
            sumcoef = spool.tile([128, NT * G], f32)
            wp_gp = AP(tensor=wp.tensor, offset=wp[:, :].offset,
                       ap=[[wp[:, :].ap[0][0], 128], [P * G, NT], [1, G], [G, P]])
            nc.vector.tensor_reduce(out=sumcoef[:, :].rearrange("p (t g) -> p t g", g=G),
                                    in_=wp_gp, axis=Ax.X, op=Alu.add)

            # ---- phase C: gather chunks + combine ----
            val_src = AP(tensor=valscr2.tensor, offset=valscr2[:, :].offset,
                         ap=[[2 * C, HW], [1, 4 * C]])

            for ch, (t0, ntile) in enumerate(CHUNKS):
                nidx = ntile * P * 128
                gt = gpool.tile([128, TCH * P, 4 * C], bf16, tag="g", bufs=3,
                                name=f"g{ch}")
                nc.gpsimd.dma_gather(
                    gt[:, : ntile * P, :], val_src,
                    idx16[:, COLOFF[ch] : COLOFF[ch] + nidx // 16],
                    num_idxs=nidx, num_idxs_reg=nidx,
                    elem_size=4 * C, elem_step=2 * C, single_packet=False,
                )

                for tt_ in range(ntile):
                    t = t0 + tt_
                    # tp[q, (j36, c)] = gt * kw  (j36 = (pt, x, yy), c = (g, gc))
                    tp = tpool.tile([128, P * 4 * C], bf16, tag="tp", bufs=1,
                                    name=f"tp{t}")
                    g_v = AP(tensor=gt.tensor,
                             offset=gt[:, :, :].offset + tt_ * P * 4 * C,
                             ap=[[gt[:, :, :].ap[0][0], 128],
                                 [C, P * 4], [GC, G], [1, GC]])
                    kw_v = AP(tensor=kw.tensor, offset=kw[:, :].offset + t * P * 4 * G,
                              ap=[[kw[:, :].ap[0][0], 128],
                                  [G, P * 4], [1, G], [0, GC]])
                    tp_v = AP(tensor=tp.tensor, offset=tp[:, :].offset,
                              ap=[[tp[:, :].ap[0][0], 128],
                                  [C, P * 4], [GC, G], [1, GC]])
                    nc.vector.tensor_tensor(out=tp_v, in0=g_v, in1=kw_v, op=Alu.mult)

                    # pre-add yy pairs (2x mode): tp2[q, ((pt,x), c)]
                    tp2 = tpool.tile([128, P * 2 * C], bf16, tag="tp2", name=f"tp2{t}")
                    in0 = AP(tensor=tp.tensor, offset=tp[:, :].offset,
                             ap=[[tp[:, :].ap[0][0], 128], [2 * C, 2 * P], [1, C]])
                    in1 = AP(tensor=tp.tensor, offset=tp[:, :].offset + C,
                             ap=[[tp[:, :].ap[0][0], 128], [2 * C, 2 * P], [1, C]])
                    out2 = AP(tensor=tp2.tensor, offset=tp2[:, :].offset,
                              ap=[[tp2[:, :].ap[0][0], 128], [C, 2 * P], [1, C]])
                    nc.vector.tensor_tensor(out=out2, in0=in0, in1=in1, op=Alu.add)

                    # ebias[q, c] = value_b[c] * sumcoef[q, g(c)]  (bf16)
                    ebias = apool.tile([128, C], bf16, tag="eb", bufs=1, name=f"eb{t}")
                    sc_v = AP(tensor=sumcoef.tensor,
                              offset=sumcoef[:, :].offset + t * G,
                              ap=[[sumcoef[:, :].ap[0][0], 128], [1, G], [0, GC]])
                    bv_v = bvr_s[:, :].rearrange("p (g c) -> p g c", g=G)
                    nc.vector.tensor_tensor(
                        out=ebias[:, :].rearrange("p (g c) -> p g c", g=G),
                        in0=sc_v, in1=bv_v, op=Alu.mult)

                    # 19 transpose-accumulate matmuls: aggT[c, q] in PSUM
                    aggT = pspool.tile([128, 128], f32, tag="mm128", bufs=2,
                                       name=f"aggT{t}")
                    for k in range(2 * P):
                        nc.tensor.matmul(aggT[:], tp2[:, k * C : (k + 1) * C],
                                         ident_s[:], start=(k == 0), stop=False)
                    nc.tensor.matmul(aggT[:], ebias[:], ident_s[:],
                                     start=False, stop=True)

                    # out-projection straight off the accumulated PSUM
                    aT = apool.tile([128, 128], bf16, tag="aT", bufs=1, name=f"aT{t}")
                    nc.scalar.copy(aT[:], aggT[:])
                    fops = pspool.tile([128, C], f32, tag="mm128", bufs=2,
                                       name=f"fo{t}")
                    nc.tensor.matmul(fops[:], owT_s[:], aT[:], start=True, stop=True)
                    fo_sb = apool.tile([128, C], f32, tag="fosb", name=f"fosb{t}")
                    nc.scalar.activation(fo_sb[:], fops[:], Act.Identity,
                                         bias=outb_s[:, 0:1], scale=1.0)
                    nc.sync.dma_start(out_d[:, t * 128 : (t + 1) * 128], fo_sb[:])

    nc.finalize()
    return nc


def _host_prep(inputs):
    """Prepare per-core input maps from full inputs."""
    feats = np.asarray(inputs["feats"], np.float32)          # [B, C, H, W]
    anchor = np.asarray(inputs["anchor_points"], np.float32)  # [B, HW, 2]
    value_w = np.asarray(inputs["value_w"], np.float32)
    value_b = np.asarray(inputs["value_b"], np.float32)
    weights_w = np.asarray(inputs["weights_w"], np.float32)
    weights_b = np.asarray(inputs["weights_b"], np.float32)
    offset_w = np.asarray(inputs["offset_w"], np.float32)
    offset_b = np.asarray(inputs["offset_b"], np.float32)
    out_w = np.asarray(inputs["out_w"], np.float32)
    out_b = np.asarray(inputs["out_b"], np.float32)

    shared = {
        "vwT16": np.ascontiguousarray(value_w.T).astype(ml_dtypes.bfloat16),
        "w72T": np.ascontiguousarray(weights_w.T).astype(ml_dtypes.bfloat16),
        "w18T": np.ascontiguousarray(offset_w.T),
        "owT16": np.ascontiguousarray(out_w.T).astype(ml_dtypes.bfloat16),
        "b72r": np.broadcast_to(weights_b, (128, 72)).copy(),
        "b18r": np.broadcast_to(offset_b, (128, 18)).copy(),
        "bvr": np.broadcast_to(value_b, (128, C)).copy(),
        "outb": out_b.reshape(128, 1).copy(),
        "ident16": np.eye(128, dtype=ml_dtypes.bfloat16),
    }
    oneh = np.zeros((128, 8, 128), np.float32)
    for qh in range(8):
        for m in range(128):
            oneh[16 * qh + (m % 16), qh, m] = 1.0
    shared["oneh"] = oneh.reshape(128, 8 * 128)

    in_maps = []
    feats16_b = [feats[b].reshape(C, HW).astype(ml_dtypes.bfloat16) for b in range(B)]
    for core in range(NCORES):
        b_i, sl = core // 4, core % 4
        off = sl * QS
        f16 = np.roll(feats16_b[b_i], -off, axis=1)
        f32r = np.roll(feats[b_i].reshape(C, HW), -off, axis=1)[:, :QS]
        an = anchor[b_i, off : off + QS].reshape(NT, 128, 2).transpose(1, 0, 2).reshape(128, NT * 2)
        m = dict(shared)
        m["feats16"] = np.ascontiguousarray(f16)
        m["feats32"] = np.ascontiguousarray(f32r)
        m["anch"] = np.ascontiguousarray(an)
        m["rotoff"] = np.full((128, 1), float(off), np.float32)
        in_maps.append(m)
    return in_maps


def kernel(**inputs) -> np.ndarray:
    from concourse.bass_utils import run_bass_kernel_spmd

    if "nc" not in _CACHE:
        _CACHE["nc"] = _build_nc()
    nc = _CACHE["nc"]
    in_maps = _host_prep(inputs)
    res = run_bass_kernel_spmd(nc, in_maps, core_ids=list(range(NCORES)))
    out = np.zeros((B, C, HW), np.float32)
    for core in range(NCORES):
        b_i, sl = core // 4, core % 4
        out[b_i, :, sl * QS : (sl + 1) * QS] = res.results[core]["out"]
    return out.reshape(B, C, H, W)
